# revision 39
# baseline (speedup 1.0000x reference)
"""Causal GQA attention block (QK L2-norm + RoPE) for 8 trn2 NeuronCores.

Sharding: tensor-parallel over head-halves (2) x data-parallel over batch (4).
Core c handles batch c//2 and heads [h*8, h*8+8) with h = c%2.

Fast-path design:
  - QK projection in fp8e4m3 DoubleRow (256-deep contraction per pass).
    Weights pre-scaled by 64; the L2 norm absorbs the scale exactly.
  - Linear softmax: with QK-norm the logits are bounded by +-0.0884, so
    exp(p) ~ 1 + p.  att = 1 + SCALE*s splits into an exact base (cumsum
    tables) plus an fp8 correction d8 (x) v8 run as DoubleRow matmuls.
  - Base numerator lam*cumsum(V^T) comes from a DVE prefix scan over
    PE-transposed V (no triangle matmuls); the denominator row-sum uses
    sum_k s_jk = q_j . cumsum(K)_j (one DVE mul + one ones-matmul per
    q-tile) instead of per-pair fp8 sum matmuls.
  - Diagonal score/AV matmuls run only over their causal column ranges.
  - Phase-1 norm/rope chain is pipelined across iterations: the ssq/swap
    matmuls of iteration i-1 are emitted after the raw matmuls of i, so
    the tensor queue never waits on the scalar/vector chain.
  - Phase-2 processes q-tiles in interleaved pairs (3,0) then (2,1) so
    the vector-heavy small tiles hide under the tensor-heavy large ones.
All scale factors are folded into host tables (lambda = 1/SCALE).
"""

import numpy as np
import ml_dtypes

import concourse.mybir as mybir
import concourse.tile as tile
from concourse import bacc
from concourse import bass2jax

F32 = mybir.dt.float32
F32R = mybir.dt.float32r
BF16 = mybir.dt.bfloat16
F8 = mybir.dt.float8e4
AF = mybir.ActivationFunctionType
ALU = mybir.AluOpType
PM = mybir.MatmulPerfMode

NPF8 = ml_dtypes.float8_e4m3
NPBF = ml_dtypes.bfloat16

P = 128
B, T, D = 4, 2048, 2048
N_HEADS, HEAD_DIM, N_KV = 16, 128, 4
Q_DIM = N_HEADS * HEAD_DIM          # 2048
KV_DIM = N_KV * HEAD_DIM            # 512
H_Q = 8                             # q heads per core
H_KV = 2                            # kv heads per core
EQ = H_Q * HEAD_DIM                 # 1024 q features per core
EKV = H_KV * HEAD_DIM               # 256
SCALE = 0.08838834764831845
LAM = 1.0 / SCALE
WSCALE = 64.0                       # fp8 pre-scale on w_qk (norm absorbs it)
THETA = 10000.0

KSUB = D // P                       # 16 contraction subtiles
KPAIR = KSUB // 2                   # 8 DoubleRow pairs
N_CORES = 8
TT_HALF = T // 2                    # 1024, phase-1 token half
NT512 = T // 512                    # 4 512-token q tiles
NTB = T // P                        # 16 128-token blocks


def _build_module():
    nc = bacc.Bacc("TRN2", target_bir_lowering=False, debug=False)

    x16t = nc.dram_tensor("x16t", [D, T], BF16, kind="ExternalInput")
    x8t = nc.dram_tensor("x8t", [D, T], F8, kind="ExternalInput")
    wq8 = nc.dram_tensor("wq8", [H_Q, P, KSUB, P], F8, kind="ExternalInput")
    wk8 = nc.dram_tensor("wk8", [P, KSUB, EKV], F8, kind="ExternalInput")
    wv16 = nc.dram_tensor("wv16", [P, KSUB, EKV], BF16, kind="ExternalInput")
    wo16 = nc.dram_tensor("wo16", [P, H_Q, D], BF16, kind="ExternalInput")
    cos_t = nc.dram_tensor("cos_t", [P, T], BF16, kind="ExternalInput")
    sin_t = nc.dram_tensor("sin_t", [P, T], BF16, kind="ExternalInput")
    ones_m = nc.dram_tensor("ones_m", [P, P], BF16, kind="ExternalInput")
    pswap = nc.dram_tensor("pswap", [P, P], BF16, kind="ExternalInput")
    ident = nc.dram_tensor("ident", [P, P], BF16, kind="ExternalInput")
    mask_t = nc.dram_tensor("mask_t", [P, 2, 512], F32R, kind="ExternalInput")
    iota_t = nc.dram_tensor("iota_t", [P, T], F32, kind="ExternalInput")
    out_t = nc.dram_tensor("out_t", [D, T], BF16, kind="ExternalOutput")

    with tile.TileContext(nc) as tc:
        with (
            tc.tile_pool(name="persist", bufs=1) as persist,
            tc.tile_pool(name="kv_persist", bufs=1) as kvp,
            tc.tile_pool(name="att_sb", bufs=8) as att_sb,
        ):
            ones_sb = persist.tile([P, P], BF16)
            psw_sb = persist.tile([P, P], BF16)
            id_sb = persist.tile([P, P], BF16)
            mask_sb = persist.tile([P, 2, 512], F32R)
            iota_sb = persist.tile([P, T], F32)
            nc.gpsimd.dma_start(ones_sb[:], ones_m.ap())
            nc.gpsimd.dma_start(psw_sb[:], pswap.ap())
            nc.gpsimd.dma_start(id_sb[:], ident.ap())
            k_sb = kvp.tile([P, H_KV, T], BF16)     # roped+normed K^T slabs
            v8_sb = kvp.tile([P, NTB, EKV], F8)     # V in [t, e] layout, fp8
            vT_sb = kvp.tile([P, H_KV, T], BF16)    # V^T in [e, t]
            q_all = kvp.tile([P, H_Q, T], BF16)     # Q resident in SBUF
            kc_sb = kvp.tile([P, H_KV, T], BF16)    # cumsum(K) along tokens
            vc_sb = kvp.tile([P, H_KV, T], F32)     # cumsum(V^T)

            # ---------------- phase 1: qkv proj + L2 norm + rope ----------
            with (
                tc.tile_pool(name="xres", bufs=1) as xres,
                tc.tile_pool(name="wstream", bufs=3) as wstream,
                tc.tile_pool(name="wvres", bufs=1) as wvres,
                tc.tile_pool(name="p1tmp", bufs=3) as p1tmp,
                tc.tile_pool(name="vstage", bufs=2) as vstage,
                tc.tile_pool(name="trig", bufs=1) as trig,
                tc.tile_pool(name="pp", bufs=4, space="PSUM") as pp,
                tc.tile_pool(name="pssq", bufs=2, space="PSUM") as pssq,
                tc.tile_pool(name="psw", bufs=2, space="PSUM") as psw,
            ):
                cos_sb = trig.tile([P, T], BF16)
                sin_sb = trig.tile([P, T], BF16)
                wv_sb = wvres.tile([P, KSUB, EKV], BF16)
                wk_sb = wvres.tile([P, KSUB, EKV], F8, name="wk_sb")
                nc.sync.dma_start(wk_sb[:, :, :P], wk8.ap()[:, :, :P])
                nc.gpsimd.dma_start(wk_sb[:, :, P:], wk8.ap()[:, :, P:])

                # two-stage deferred tail of the norm/rope chain: the ssq/
                # norm DVE chain runs one iteration behind the raw matmuls,
                # the swap/rope matmuls three behind, so no engine ever waits
                # on a cross-engine result emitted in the same iteration
                p1q = []
                p2q = []

                def emit_part1(c):
                    raw_ps, sq, dest, tg = c
                    ssq_ps = pssq.tile([P, 512], F32, tag="ssq")
                    nc.tensor.matmul(
                        ssq_ps[:], ones_sb[:], sq[:], start=True, stop=True
                    )
                    s_sb = p1tmp.tile([P, 512], F32, tag="t2")
                    nc.scalar.activation(s_sb[:], ssq_ps[:], AF.Sqrt)
                    r_sb = p1tmp.tile([P, 512], F32, tag="t3")
                    nc.vector.reciprocal_approx_fast(r_sb[:], s_sb[:])
                    qn = p1tmp.tile([P, 512], BF16, tag="t4")
                    nc.vector.tensor_mul(qn[:], raw_ps[:], r_sb[:])
                    ys = p1tmp.tile([P, 512], BF16, tag="t5")
                    nc.vector.tensor_mul(ys[:], qn[:], sin_sb[:, tg : tg + 512])
                    qc = p1tmp.tile([P, 512], BF16, tag="t6")
                    nc.vector.tensor_mul(qc[:], qn[:], cos_sb[:, tg : tg + 512])
                    return (ys, qc, dest)

                def emit_part2(c):
                    ys, qc, dest = c
                    sw_ps = psw.tile([P, 512], F32, tag="sw")
                    nc.tensor.matmul(
                        sw_ps[:], psw_sb[:], ys[:], start=True, stop=False
                    )
                    nc.tensor.matmul(
                        sw_ps[:], id_sb[:], qc[:], start=False, stop=True
                    )
                    nc.scalar.copy(dest, sw_ps[:])

                def step():
                    if len(p1q) > 1:
                        p2q.append(emit_part1(p1q.pop(0)))
                    if len(p2q) > 2:
                        emit_part2(p2q.pop(0))

                def flush():
                    while p1q:
                        p2q.append(emit_part1(p1q.pop(0)))
                    while p2q:
                        emit_part2(p2q.pop(0))

                for th in range(2):
                    t0 = th * TT_HALF
                    x8_sb = [
                        xres.tile([P, 2, TT_HALF], F8, tag=f"x8_{kp}", name=f"x8_{kp}")
                        for kp in range(KPAIR)
                    ]
                    x16_sb = [
                        xres.tile([P, TT_HALF], BF16, tag=f"x16_{ks}", name=f"x16_{ks}")
                        for ks in range(KSUB)
                    ]
                    xr16 = x16t.ap()[:, t0 : t0 + TT_HALF].rearrange(
                        "(ks p) t -> p ks t", p=P
                    )
                    xr8 = x8t.ap()[:, t0 : t0 + TT_HALF].rearrange(
                        "(ks p) t -> p ks t", p=P
                    )
                    # x8 first (pair 0 gates the first raw matmul): even ks
                    # on sync, odd ks on gpsimd; scalar issues no DMAs so the
                    # norm-chain Squares never queue behind transfers.  One
                    # 2D DMA per ks slab (a fused [P,2,T/2] copy would be 3D).
                    for kp in range(KPAIR):
                        nc.sync.dma_start(x8_sb[kp][:, 0], xr8[:, 2 * kp])
                        nc.gpsimd.dma_start(x8_sb[kp][:, 1], xr8[:, 2 * kp + 1])
                    if th == 0:
                        nc.gpsimd.dma_start(cos_sb[:], cos_t.ap())
                        nc.gpsimd.dma_start(sin_sb[:], sin_t.ap())
                    for ks in range(KSUB):
                        nc.gpsimd.dma_start(x16_sb[ks][:], xr16[:, ks])
                    if th == 0:
                        nc.gpsimd.dma_start(wv_sb[:], wv16.ap())
                    else:
                        # phase-2 tables, needed right at the boundary
                        nc.sync.dma_start(mask_sb[:], mask_t.ap())
                        nc.sync.dma_start(iota_sb[:], iota_t.ap())

                    def proj_norm_rope(es):
                        """project feature block es (fp8 DoubleRow), norm, rope"""
                        if es < H_Q:
                            w_sb = wstream.tile([P, KSUB, P], F8, tag="w")
                            nc.sync.dma_start(w_sb[:], wq8.ap()[es])
                            w_use = w_sb
                        else:
                            w_use = wk_sb
                        for tt in range(2):
                            tg = t0 + tt * 512
                            sl = slice(tt * 512, (tt + 1) * 512)
                            raw_ps = pp.tile([P, 512], F32, tag="raw")
                            for kp in range(KPAIR):
                                if es < H_Q:
                                    lhs = w_use[:, 2 * kp : 2 * kp + 2, :]
                                else:
                                    e0 = (es - H_Q) * P
                                    lhs = w_use[:, 2 * kp : 2 * kp + 2, e0 : e0 + P]
                                nc.tensor.matmul(
                                    raw_ps[:],
                                    lhs,
                                    x8_sb[kp][:, :, sl],
                                    start=(kp == 0),
                                    stop=(kp == KPAIR - 1),
                                    perf_mode=PM.DoubleRow,
                                )
                            sq = p1tmp.tile([P, 512], BF16, tag="t1")
                            nc.scalar.activation(sq[:], raw_ps[:], AF.Square)
                            if es < H_Q:
                                dest = q_all[:, es, tg : tg + 512]
                            else:
                                dest = k_sb[:, es - H_Q, tg : tg + 512]
                            p1q.append((raw_ps, sq, dest, tg))
                            step()

                    # K first so downstream work can start earliest, then Q
                    for es in (H_Q, H_Q + 1):
                        proj_norm_rope(es)
                    if th == 1:
                        # K complete: start its prefix scan now so phase 2
                        # isn't gated on the end of the vector queue
                        flush()
                        for kvi in range(H_KV):
                            nc.vector.tensor_tensor_scan(
                                kc_sb[:, kvi],
                                k_sb[:, kvi],
                                k_sb[:, kvi],
                                0.0,
                                ALU.add,
                                ALU.bypass,
                            )
                    for es in range(H_Q):
                        proj_norm_rope(es)
                    flush()
                    # V projection + fp8 cast + PE transpose for the scan
                    for tb in range(TT_HALF // P):
                        tbg = th * (TT_HALF // P) + tb
                        v_ps = pp.tile([P, EKV], F32, tag="raw")
                        for ks in range(KSUB):
                            nc.tensor.matmul(
                                v_ps[:],
                                x16_sb[ks][:, tb * P : (tb + 1) * P],
                                wv_sb[:, ks],
                                start=(ks == 0),
                                stop=(ks == KSUB - 1),
                            )
                        vst = vstage.tile([P, EKV], BF16, tag="vs")
                        nc.scalar.copy(vst[:], v_ps[:])
                        nc.scalar.copy(v8_sb[:, tbg], v_ps[:])
                        for kvi in range(H_KV):
                            tp_ps = pssq.tile([P, P], BF16, tag="ssq")
                            nc.tensor.transpose(
                                tp_ps[:],
                                vst[:, kvi * P : (kvi + 1) * P],
                                id_sb[:],
                            )
                            nc.scalar.copy(
                                vT_sb[:, kvi, tbg * P : (tbg + 1) * P], tp_ps[:]
                            )
                # V^T complete: full-length prefix scan
                for kvi in range(H_KV):
                    nc.vector.tensor_tensor_scan(
                        vc_sb[:, kvi],
                        vT_sb[:, kvi],
                        vT_sb[:, kvi],
                        0.0,
                        ALU.add,
                        ALU.bypass,
                    )

            # ------- phase 2: attention + output projection per q-tile ----
            with (
                tc.tile_pool(name="wores", bufs=1) as wores,
                tc.tile_pool(name="p2tmp", bufs=4) as p2tmp,
                tc.tile_pool(name="oall", bufs=2) as oall,
                tc.tile_pool(name="fout", bufs=3) as fout,
                tc.tile_pool(name="psc", bufs=4, space="PSUM") as psc,
                tc.tile_pool(name="pav", bufs=2, space="PSUM") as pav,
                tc.tile_pool(name="psum2", bufs=2, space="PSUM") as psum2,
            ):
                wo_sb = wores.tile([P, H_Q, D], BF16)
                for ei in range(H_Q):
                    nc.gpsimd.dma_start(wo_sb[:, ei], wo16.ap()[:, ei])

                def attn_head(qt, hd, o_all):
                    q0 = qt * 512
                    nkb = (qt + 1) * 4
                    npair = nkb // 2
                    kvi = hd // 4
                    o_ps = pav.tile([P, 512], F32, tag="av")

                    # denominator numerand: q . Kc (exact causal row-sum)
                    qkc = p2tmp.tile([P, 512], BF16, tag="qk")
                    nc.gpsimd.tensor_mul(
                        qkc[:],
                        q_all[:, hd, q0 : q0 + 512],
                        kc_sb[:, kvi, q0 : q0 + 512],
                    )
                    # all scores + casts first (per-block psum tiles, 4-deep
                    # rotation hides cast latency), then all AV matmuls
                    d8s = []
                    for pj in range(npair - 2):
                        d8 = att_sb.tile([P, 2, 512], F8, tag="att")
                        for j in range(2):
                            kb = 2 * pj + j
                            sc_ps = psc.tile([P, 512], F32, tag="sc")
                            nc.tensor.matmul(
                                sc_ps[:],
                                k_sb[:, kvi, kb * P : (kb + 1) * P],
                                q_all[:, hd, q0 : q0 + 512],
                                start=True,
                                stop=True,
                            )
                            nc.scalar.activation(d8[:, j], sc_ps[:], AF.Copy)
                        d8s.append(d8)
                    # diagonal pair A (rel blocks 0,1): masked, full width
                    d8a = att_sb.tile([P, 2, 512], F8, tag="att")
                    for j in range(2):
                        sc_ps = psc.tile([P, 512], F32, tag="sc")
                        nc.tensor.matmul(
                            sc_ps[:],
                            k_sb[:, kvi, (nkb - 4 + j) * P : (nkb - 3 + j) * P],
                            q_all[:, hd, q0 : q0 + 512],
                            start=True,
                            stop=True,
                        )
                        nc.vector.tensor_mul(d8a[:, j], sc_ps[:], mask_sb[:, j])
                    # diagonal pair B (rel blocks 2,3): cols [256:512)
                    d8b = att_sb.tile([P, 2, 256], F8, tag="attb")
                    for j in range(2):
                        sc_ps = psc.tile([P, 256], F32, tag="sc")
                        nc.tensor.matmul(
                            sc_ps[:],
                            k_sb[:, kvi, (nkb - 2 + j) * P : (nkb - 1 + j) * P],
                            q_all[:, hd, q0 + 256 : q0 + 512],
                            start=True,
                            stop=True,
                        )
                        nc.vector.tensor_mul(
                            d8b[:, j], sc_ps[:], mask_sb[:, j, :256]
                        )
                    for pj in range(npair - 2):
                        kb0 = 2 * pj
                        nc.tensor.matmul(
                            o_ps[:],
                            v8_sb[:, kb0 : kb0 + 2, kvi * P : (kvi + 1) * P],
                            d8s[pj][:],
                            start=(pj == 0),
                            stop=False,
                            perf_mode=PM.DoubleRow,
                        )
                    nc.tensor.matmul(
                        o_ps[:],
                        v8_sb[:, nkb - 4 : nkb - 2, kvi * P : (kvi + 1) * P],
                        d8a[:],
                        start=(npair == 2),
                        stop=True,
                        perf_mode=PM.DoubleRow,
                    )
                    nc.tensor.matmul(
                        o_ps[:, 256:],
                        v8_sb[:, nkb - 2 : nkb, kvi * P : (kvi + 1) * P],
                        d8b[:],
                        start=False,
                        stop=True,
                        perf_mode=PM.DoubleRow,
                        skip_group_check=True,
                    )
                    # denominator: iota + sum(q . Kc)
                    den_ps = psum2.tile([P, 512], F32, tag="sum")
                    nc.tensor.matmul(
                        den_ps[:], ones_sb[:], qkc[:], start=True, stop=True
                    )
                    den = p2tmp.tile([P, 512], F32, tag="dn")
                    nc.vector.tensor_add(
                        den[:], den_ps[:], iota_sb[:, q0 : q0 + 512]
                    )
                    rs = p2tmp.tile([P, 512], F32, tag="rs")
                    nc.vector.reciprocal_approx_fast(rs[:], den[:])
                    # finalize: (corr + lam*Vc) / den
                    fa = p2tmp.tile([P, 512], BF16, tag="fa")
                    nc.vector.scalar_tensor_tensor(
                        fa[:],
                        vc_sb[:, kvi, q0 : q0 + 512],
                        float(LAM),
                        o_ps[:],
                        op0=ALU.mult,
                        op1=ALU.add,
                    )
                    nc.gpsimd.tensor_mul(o_all[:, hd], fa[:], rs[:])

                def o_proj(qt, o_all):
                    q0 = qt * 512
                    for eo in range(D // P):
                        f_ps = psum2.tile([P, 512], F32, tag="sum")
                        for ei in range(H_Q):
                            nc.tensor.matmul(
                                f_ps[:],
                                wo_sb[:, ei, eo * P : (eo + 1) * P],
                                o_all[:, ei],
                                start=(ei == 0),
                                stop=(ei == H_Q - 1),
                            )
                        f_sb = fout.tile([P, 512], BF16, tag="fo")
                        nc.scalar.copy(f_sb[:], f_ps[:])
                        nc.sync.dma_start(
                            out_t.ap()[eo * P : (eo + 1) * P, q0 : q0 + 512],
                            f_sb[:],
                        )

                # interleave a tensor-heavy and a vector-heavy q-tile so the
                # small tiles' DVE chains hide under the big tiles' matmuls
                for qta, qtb in ((3, 0), (2, 1)):
                    oa = oall.tile([P, H_Q, 512], BF16, tag="oa", name="oa_a")
                    ob = oall.tile([P, H_Q, 512], BF16, tag="oa", name="oa_b")
                    for hd in range(H_Q):
                        attn_head(qta, hd, oa)
                        attn_head(qtb, hd, ob)
                    o_proj(qta, oa)
                    o_proj(qtb, ob)

    nc.compile()
    return nc


def _re3(a):
    """[K, E] -> [P, K//P, E] host rearrange for contiguous weight DMAs."""
    return np.ascontiguousarray(a.reshape(-1, P, a.shape[1]).transpose(1, 0, 2))


def _host_inputs(x, w_qkv, w_o):
    """Build the 8 per-core input maps from full inputs."""
    x = np.asarray(x, dtype=np.float32)
    w_qkv = np.asarray(w_qkv, dtype=np.float32)
    w_o = np.asarray(w_o, dtype=np.float32)

    half = HEAD_DIM // 2
    inv_freq = 1.0 / (
        THETA ** (np.arange(0, HEAD_DIM, 2, dtype=np.float32) / HEAD_DIM)
    )
    ang = np.arange(T, dtype=np.float32)[:, None] * inv_freq[None, :]  # [T, 64]
    cos = np.cos(ang).T.astype(np.float32)  # [64, T]
    sin = np.sin(ang).T.astype(np.float32)
    cos_t = np.ascontiguousarray(np.concatenate([cos, cos], axis=0)).astype(NPBF)
    sin_t = np.ascontiguousarray(np.concatenate([sin, sin], axis=0)).astype(NPBF)

    ones_m = np.ones((P, P), dtype=np.float32).astype(NPBF)
    pswap = np.zeros((P, P), dtype=np.float32)
    for p in range(half):
        pswap[p, p + half] = 1.0    # out[m=p+64] += ys[p]
        pswap[p + half, p] = -1.0   # out[m=p]    -= ys[p+64]
    pswap = pswap.astype(NPBF)
    ident = np.eye(P, dtype=np.float32).astype(NPBF)

    t_idx = np.arange(P, dtype=np.float32)[:, None]        # key within block
    j_idx = np.arange(512, dtype=np.float32)[None, :]      # query col
    mask_t = np.zeros((P, 2, 512), dtype=np.float32)
    for s_ in range(2):
        mask_t[:, s_] = 1.0 * (t_idx <= j_idx - 128 * s_)
    iota_t = np.broadcast_to(
        (np.arange(T, dtype=np.float32) + 1.0) * np.float32(LAM), (P, T)
    ).copy()

    in_maps = []
    for c in range(N_CORES):
        b, h = c // 2, c % 2
        qrows = slice(h * EQ, (h + 1) * EQ)
        krows = slice(Q_DIM + h * EKV, Q_DIM + (h + 1) * EKV)
        vrows = slice(Q_DIM + KV_DIM + h * EKV, Q_DIM + (h + 1) * EKV + KV_DIM)
        wq_r = _re3(np.ascontiguousarray(w_qkv[qrows].T * WSCALE))
        wq_r4 = np.ascontiguousarray(
            wq_r.reshape(P, KSUB, H_Q, P).transpose(2, 0, 1, 3)
        ).astype(NPF8)  # [H_Q, P, 16, 128]
        xt = np.ascontiguousarray(x[b].T)
        in_maps.append(
            {
                "x16t": xt.astype(NPBF),
                "x8t": xt.astype(NPF8),
                "wq8": wq_r4,
                "wk8": _re3(np.ascontiguousarray(w_qkv[krows].T * WSCALE)).astype(
                    NPF8
                ),
                "wv16": _re3(np.ascontiguousarray(w_qkv[vrows].T)).astype(NPBF),
                "wo16": _re3(
                    np.ascontiguousarray(w_o[:, h * EQ : (h + 1) * EQ].T)
                ).reshape(P, H_Q, D).astype(NPBF),
                "cos_t": cos_t,
                "sin_t": sin_t,
                "ones_m": ones_m,
                "pswap": pswap,
                "ident": ident,
                "mask_t": mask_t,
                "iota_t": iota_t,
            }
        )
    return in_maps


def _gather(results):
    out = np.empty((B, T, D), dtype=np.float32)
    for b in range(B):
        acc = np.asarray(results[2 * b]["out_t"], np.float32) + np.asarray(
            results[2 * b + 1]["out_t"], np.float32
        )
        out[b] = acc.T
    return out


_NC_CACHE = []


def _get_module():
    if not _NC_CACHE:
        _NC_CACHE.append(_build_module())
    return _NC_CACHE[0]


def kernel(x, w_qkv, w_o):
    nc = _get_module()
    in_maps = _host_inputs(x, w_qkv, w_o)
    results = bass2jax.run_bass_via_pjrt(nc, in_maps, n_cores=N_CORES)
    return _gather(results)


# revision 45
# speedup vs baseline: 1.0678x; 1.0678x over previous
"""Causal GQA attention block (QK L2-norm + RoPE) for 8 trn2 NeuronCores.

Sharding: tensor-parallel over head-halves (2) x data-parallel over batch (4).
Core c handles batch c//2 and heads [h*8, h*8+8) with h = c%2.

Fast-path design:
  - QK projection in fp8e4m3 DoubleRow (256-deep contraction per pass).
    Weights pre-scaled by 64; the L2 norm absorbs the scale exactly.
  - Linear softmax: with QK-norm the logits are bounded by +-0.0884, so
    exp(p) ~ 1 + p.  att = 1 + SCALE*s splits into an exact base (cumsum
    tables) plus an fp8 correction d8 (x) v8 run as DoubleRow matmuls.
  - Base numerator lam*cumsum(V^T) comes from a DVE prefix scan over
    PE-transposed V (no triangle matmuls); the denominator row-sum uses
    sum_k s_jk = q_j . cumsum(K)_j (one DVE mul + one ones-matmul per
    q-tile) instead of per-pair fp8 sum matmuls.
  - Diagonal score/AV matmuls run only over their causal column ranges.
  - Phase-1 norm/rope chain is pipelined across iterations: the ssq/swap
    matmuls of iteration i-1 are emitted after the raw matmuls of i, so
    the tensor queue never waits on the scalar/vector chain.
  - Phase-2 processes q-tiles in interleaved pairs (3,0) then (2,1) so
    the vector-heavy small tiles hide under the tensor-heavy large ones.
All scale factors are folded into host tables (lambda = 1/SCALE).
"""

import numpy as np
import ml_dtypes

import concourse.mybir as mybir
import concourse.tile as tile
from concourse import bacc
from concourse import bass2jax

F32 = mybir.dt.float32
F32R = mybir.dt.float32r
BF16 = mybir.dt.bfloat16
F8 = mybir.dt.float8e4
AF = mybir.ActivationFunctionType
ALU = mybir.AluOpType
PM = mybir.MatmulPerfMode

NPF8 = ml_dtypes.float8_e4m3
NPBF = ml_dtypes.bfloat16

P = 128
B, T, D = 4, 2048, 2048
N_HEADS, HEAD_DIM, N_KV = 16, 128, 4
Q_DIM = N_HEADS * HEAD_DIM          # 2048
KV_DIM = N_KV * HEAD_DIM            # 512
H_Q = 8                             # q heads per core
H_KV = 2                            # kv heads per core
EQ = H_Q * HEAD_DIM                 # 1024 q features per core
EKV = H_KV * HEAD_DIM               # 256
SCALE = 0.08838834764831845
LAM = 1.0 / SCALE
WSCALE = 64.0                       # fp8 pre-scale on w_qk (norm absorbs it)
THETA = 10000.0

KSUB = D // P                       # 16 contraction subtiles
KPAIR = KSUB // 2                   # 8 DoubleRow pairs
N_CORES = 8
TT_HALF = T // 2                    # 1024, phase-1 token half
NT512 = T // 512                    # 4 512-token q tiles
NTB = T // P                        # 16 128-token blocks


def _build_module():
    nc = bacc.Bacc("TRN2", target_bir_lowering=False, debug=False)

    x16t = nc.dram_tensor("x16t", [D, T], BF16, kind="ExternalInput")
    x8t = nc.dram_tensor("x8t", [D, T], F8, kind="ExternalInput")
    wq8 = nc.dram_tensor("wq8", [H_Q, P, KSUB, P], F8, kind="ExternalInput")
    wk8 = nc.dram_tensor("wk8", [P, KSUB, EKV], F8, kind="ExternalInput")
    wv16 = nc.dram_tensor("wv16", [P, KSUB, EKV], BF16, kind="ExternalInput")
    wo16 = nc.dram_tensor("wo16", [P, H_Q, D], BF16, kind="ExternalInput")
    cos_t = nc.dram_tensor("cos_t", [P, T], BF16, kind="ExternalInput")
    sin_t = nc.dram_tensor("sin_t", [P, T], BF16, kind="ExternalInput")
    ones_m = nc.dram_tensor("ones_m", [P, P], BF16, kind="ExternalInput")
    pswap = nc.dram_tensor("pswap", [P, P], BF16, kind="ExternalInput")
    ident = nc.dram_tensor("ident", [P, P], BF16, kind="ExternalInput")
    mask_t = nc.dram_tensor("mask_t", [P, 2, 512], F32R, kind="ExternalInput")
    iota_t = nc.dram_tensor("iota_t", [P, T], F32, kind="ExternalInput")
    out_t = nc.dram_tensor("out_t", [D, T], BF16, kind="ExternalOutput")

    with tile.TileContext(nc) as tc:
        with (
            tc.tile_pool(name="persist", bufs=1) as persist,
            tc.tile_pool(name="kv_persist", bufs=1) as kvp,
            tc.tile_pool(name="att_sb", bufs=8) as att_sb,
        ):
            ones_sb = persist.tile([P, P], BF16)
            psw_sb = persist.tile([P, P], BF16)
            id_sb = persist.tile([P, P], BF16)
            mask_sb = persist.tile([P, 2, 512], F32R)
            iota_sb = persist.tile([P, T], F32)
            nc.gpsimd.dma_start(ones_sb[:], ones_m.ap())
            nc.gpsimd.dma_start(psw_sb[:], pswap.ap())
            nc.gpsimd.dma_start(id_sb[:], ident.ap())
            k_sb = kvp.tile([P, H_KV, T], BF16)     # roped+normed K^T slabs
            v8_sb = kvp.tile([P, NTB, EKV], F8)     # V in [t, e] layout, fp8
            vT_sb = kvp.tile([P, H_KV, T], BF16)    # V^T in [e, t]
            q_all = kvp.tile([P, H_Q, T], BF16)     # Q resident in SBUF
            kc_sb = kvp.tile([P, H_KV, T], BF16)    # cumsum(K) along tokens
            vc_sb = kvp.tile([P, H_KV, T], BF16)    # cumsum(V^T)

            # ---------------- phase 1: qkv proj + L2 norm + rope ----------
            with (
                tc.tile_pool(name="xres", bufs=1) as xres,
                tc.tile_pool(name="wstream", bufs=3) as wstream,
                tc.tile_pool(name="wvres", bufs=1) as wvres,
                tc.tile_pool(name="p1tmp", bufs=3) as p1tmp,
                tc.tile_pool(name="vstage", bufs=2) as vstage,
                tc.tile_pool(name="trig", bufs=1) as trig,
                tc.tile_pool(name="pp", bufs=4, space="PSUM") as pp,
                tc.tile_pool(name="pssq", bufs=2, space="PSUM") as pssq,
                tc.tile_pool(name="psw", bufs=2, space="PSUM") as psw,
            ):
                cos_sb = trig.tile([P, T], BF16)
                sin_sb = trig.tile([P, T], BF16)
                wv_sb = wvres.tile([P, KSUB, EKV], BF16)
                wk_sb = wvres.tile([P, KSUB, EKV], F8, name="wk_sb")
                nc.sync.dma_start(wk_sb[:, :, :P], wk8.ap()[:, :, :P])
                nc.gpsimd.dma_start(wk_sb[:, :, P:], wk8.ap()[:, :, P:])

                # deferred tail of the norm/rope chain: emitted one (es,tt)
                # iteration later so the tensor queue always has raw matmuls
                # in front of the chain-dependent ssq/swap matmuls
                pend = []

                def emit_tail(raw_ps, sq, dest, tg):
                    ssq_ps = pssq.tile([P, 512], F32, tag="ssq")
                    nc.tensor.matmul(
                        ssq_ps[:], ones_sb[:], sq[:], start=True, stop=True
                    )
                    s_sb = p1tmp.tile([P, 512], F32, tag="t2")
                    nc.scalar.activation(s_sb[:], ssq_ps[:], AF.Sqrt)
                    r_sb = p1tmp.tile([P, 512], F32, tag="t3")
                    nc.vector.reciprocal_approx_fast(r_sb[:], s_sb[:])
                    qn = p1tmp.tile([P, 512], BF16, tag="t4")
                    nc.vector.tensor_mul(qn[:], raw_ps[:], r_sb[:])
                    ys = p1tmp.tile([P, 512], BF16, tag="t5")
                    nc.vector.tensor_mul(ys[:], qn[:], sin_sb[:, tg : tg + 512])
                    sw_ps = psw.tile([P, 512], F32, tag="sw")
                    nc.tensor.matmul(
                        sw_ps[:], psw_sb[:], ys[:], start=True, stop=True
                    )
                    qc = p1tmp.tile([P, 512], BF16, tag="t6")
                    nc.gpsimd.tensor_mul(qc[:], qn[:], cos_sb[:, tg : tg + 512])
                    nc.vector.tensor_add(dest, sw_ps[:], qc[:])

                def flush():
                    while pend:
                        emit_tail(*pend.pop(0))

                for th in range(2):
                    t0 = th * TT_HALF
                    x8_sb = [
                        xres.tile(
                            [P, 2, TT_HALF],
                            F8,
                            tag=f"x8_{kp}",
                            name=f"x8_{kp}",
                            bufs=2,
                        )
                        for kp in range(KPAIR)
                    ]
                    x16_sb = [
                        xres.tile([P, TT_HALF], BF16, tag=f"x16_{ks}", name=f"x16_{ks}")
                        for ks in range(KSUB)
                    ]
                    xr16 = x16t.ap()[:, t0 : t0 + TT_HALF].rearrange(
                        "(ks p) t -> p ks t", p=P
                    )
                    xr8 = x8t.ap()[:, t0 : t0 + TT_HALF].rearrange(
                        "(ks p) t -> p ks t", p=P
                    )
                    # x8 first (pair 0 gates the first raw matmul); x16 on the
                    # gpsimd queue (only needed by the V projection).  One 2D
                    # DMA per ks slab (a fused [P,2,T/2] copy would be 3D).
                    for kp in range(KPAIR):
                        eng = nc.sync if kp % 2 == 0 else nc.scalar
                        for j in range(2):
                            eng.dma_start(x8_sb[kp][:, j], xr8[:, 2 * kp + j])
                    if th == 0:
                        nc.gpsimd.dma_start(cos_sb[:], cos_t.ap())
                        nc.gpsimd.dma_start(sin_sb[:], sin_t.ap())
                    for ks in range(KSUB):
                        nc.gpsimd.dma_start(x16_sb[ks][:], xr16[:, ks])
                    if th == 0:
                        nc.gpsimd.dma_start(wv_sb[:], wv16.ap())
                    else:
                        # phase-2 tables, needed right at the boundary
                        nc.sync.dma_start(mask_sb[:], mask_t.ap())
                        nc.sync.dma_start(iota_sb[:], iota_t.ap())

                    def proj_norm_rope(es):
                        """project feature block es (fp8 DoubleRow), norm, rope"""
                        if es < H_Q:
                            w_sb = wstream.tile([P, KSUB, P], F8, tag="w")
                            nc.sync.dma_start(w_sb[:], wq8.ap()[es])
                            w_use = w_sb
                        else:
                            w_use = wk_sb
                        for tt in range(2):
                            tg = t0 + tt * 512
                            sl = slice(tt * 512, (tt + 1) * 512)
                            raw_ps = pp.tile([P, 512], F32, tag="raw")
                            for kp in range(KPAIR):
                                if es < H_Q:
                                    lhs = w_use[:, 2 * kp : 2 * kp + 2, :]
                                else:
                                    e0 = (es - H_Q) * P
                                    lhs = w_use[:, 2 * kp : 2 * kp + 2, e0 : e0 + P]
                                nc.tensor.matmul(
                                    raw_ps[:],
                                    lhs,
                                    x8_sb[kp][:, :, sl],
                                    start=(kp == 0),
                                    stop=(kp == KPAIR - 1),
                                    perf_mode=PM.DoubleRow,
                                )
                            sq = p1tmp.tile([P, 512], BF16, tag="t1")
                            nc.scalar.activation(sq[:], raw_ps[:], AF.Square)
                            if es < H_Q:
                                dest = q_all[:, es, tg : tg + 512]
                            else:
                                dest = k_sb[:, es - H_Q, tg : tg + 512]
                            if pend:
                                emit_tail(*pend.pop(0))
                            pend.append((raw_ps, sq, dest, tg))

                    # K first so downstream work can start earliest, then Q
                    for es in (H_Q, H_Q + 1):
                        proj_norm_rope(es)
                    if th == 1:
                        # K complete: start its prefix scan now so phase 2
                        # isn't gated on the end of the vector queue
                        flush()
                        for kvi in range(H_KV):
                            nc.vector.tensor_tensor_scan(
                                kc_sb[:, kvi],
                                k_sb[:, kvi],
                                k_sb[:, kvi],
                                0.0,
                                ALU.add,
                                ALU.bypass,
                            )
                    for es in range(H_Q):
                        proj_norm_rope(es)
                    flush()
                    # V projection + fp8 cast + PE transpose for the scan
                    for tb in range(TT_HALF // P):
                        tbg = th * (TT_HALF // P) + tb
                        v_ps = pp.tile([P, EKV], F32, tag="raw")
                        for ks in range(KSUB):
                            nc.tensor.matmul(
                                v_ps[:],
                                x16_sb[ks][:, tb * P : (tb + 1) * P],
                                wv_sb[:, ks],
                                start=(ks == 0),
                                stop=(ks == KSUB - 1),
                            )
                        vst = vstage.tile([P, EKV], BF16, tag="vs")
                        nc.scalar.copy(vst[:], v_ps[:])
                        nc.scalar.copy(v8_sb[:, tbg], v_ps[:])
                        for kvi in range(H_KV):
                            tp_ps = pssq.tile([P, P], BF16, tag="ssq")
                            nc.tensor.transpose(
                                tp_ps[:],
                                vst[:, kvi * P : (kvi + 1) * P],
                                id_sb[:],
                            )
                            nc.scalar.copy(
                                vT_sb[:, kvi, tbg * P : (tbg + 1) * P], tp_ps[:]
                            )
                # V^T complete: full-length prefix scan
                for kvi in range(H_KV):
                    nc.vector.tensor_tensor_scan(
                        vc_sb[:, kvi],
                        vT_sb[:, kvi],
                        vT_sb[:, kvi],
                        0.0,
                        ALU.add,
                        ALU.bypass,
                    )

            # ------- phase 2: attention + output projection per q-tile ----
            with (
                tc.tile_pool(name="wores", bufs=1) as wores,
                tc.tile_pool(name="p2tmp", bufs=4) as p2tmp,
                tc.tile_pool(name="oall", bufs=2) as oall,
                tc.tile_pool(name="fout", bufs=3) as fout,
                tc.tile_pool(name="psc", bufs=4, space="PSUM") as psc,
                tc.tile_pool(name="pav", bufs=2, space="PSUM") as pav,
                tc.tile_pool(name="psum2", bufs=2, space="PSUM") as psum2,
            ):
                wo_sb = wores.tile([P, H_Q, D], BF16)
                for ei in range(H_Q):
                    nc.gpsimd.dma_start(wo_sb[:, ei], wo16.ap()[:, ei])

                def attn_head(qt, hd, o_all):
                    q0 = qt * 512
                    nkb = (qt + 1) * 4
                    npair = nkb // 2
                    kvi = hd // 4
                    o_ps = pav.tile([P, 512], F32, tag="av")

                    # denominator numerand: q . Kc (exact causal row-sum)
                    qkc = p2tmp.tile([P, 512], BF16, tag="qk")
                    nc.gpsimd.tensor_mul(
                        qkc[:],
                        q_all[:, hd, q0 : q0 + 512],
                        kc_sb[:, kvi, q0 : q0 + 512],
                    )
                    # all scores + casts first (per-block psum tiles, 4-deep
                    # rotation hides cast latency), then all AV matmuls
                    d8s = []
                    for pj in range(npair - 2):
                        d8 = att_sb.tile([P, 2, 512], F8, tag="att")
                        for j in range(2):
                            kb = 2 * pj + j
                            sc_ps = psc.tile([P, 512], F32, tag="sc")
                            nc.tensor.matmul(
                                sc_ps[:],
                                k_sb[:, kvi, kb * P : (kb + 1) * P],
                                q_all[:, hd, q0 : q0 + 512],
                                start=True,
                                stop=True,
                            )
                            nc.scalar.activation(d8[:, j], sc_ps[:], AF.Copy)
                        d8s.append(d8)
                    # diagonal pair A (rel blocks 0,1): masked, full width
                    d8a = att_sb.tile([P, 2, 512], F8, tag="att")
                    for j in range(2):
                        sc_ps = psc.tile([P, 512], F32, tag="sc")
                        nc.tensor.matmul(
                            sc_ps[:],
                            k_sb[:, kvi, (nkb - 4 + j) * P : (nkb - 3 + j) * P],
                            q_all[:, hd, q0 : q0 + 512],
                            start=True,
                            stop=True,
                        )
                        nc.vector.tensor_mul(d8a[:, j], sc_ps[:], mask_sb[:, j])
                    # diagonal pair B (rel blocks 2,3): cols [256:512)
                    d8b = att_sb.tile([P, 2, 256], F8, tag="attb")
                    for j in range(2):
                        sc_ps = psc.tile([P, 256], F32, tag="sc")
                        nc.tensor.matmul(
                            sc_ps[:],
                            k_sb[:, kvi, (nkb - 2 + j) * P : (nkb - 1 + j) * P],
                            q_all[:, hd, q0 + 256 : q0 + 512],
                            start=True,
                            stop=True,
                        )
                        nc.vector.tensor_mul(
                            d8b[:, j], sc_ps[:], mask_sb[:, j, :256]
                        )
                    for pj in range(npair - 2):
                        kb0 = 2 * pj
                        nc.tensor.matmul(
                            o_ps[:],
                            v8_sb[:, kb0 : kb0 + 2, kvi * P : (kvi + 1) * P],
                            d8s[pj][:],
                            start=(pj == 0),
                            stop=False,
                            perf_mode=PM.DoubleRow,
                        )
                    nc.tensor.matmul(
                        o_ps[:],
                        v8_sb[:, nkb - 4 : nkb - 2, kvi * P : (kvi + 1) * P],
                        d8a[:],
                        start=(npair == 2),
                        stop=True,
                        perf_mode=PM.DoubleRow,
                    )
                    nc.tensor.matmul(
                        o_ps[:, 256:],
                        v8_sb[:, nkb - 2 : nkb, kvi * P : (kvi + 1) * P],
                        d8b[:],
                        start=False,
                        stop=True,
                        perf_mode=PM.DoubleRow,
                        skip_group_check=True,
                    )
                    # denominator: iota + sum(q . Kc)
                    den_ps = psum2.tile([P, 512], F32, tag="sum")
                    nc.tensor.matmul(
                        den_ps[:], ones_sb[:], qkc[:], start=True, stop=True
                    )
                    den = p2tmp.tile([P, 512], F32, tag="dn")
                    nc.vector.tensor_add(
                        den[:], den_ps[:], iota_sb[:, q0 : q0 + 512]
                    )
                    rs = p2tmp.tile([P, 512], F32, tag="rs")
                    nc.vector.reciprocal_approx_fast(rs[:], den[:])
                    # finalize: (corr + lam*Vc) / den
                    fa = p2tmp.tile([P, 512], BF16, tag="fa")
                    nc.vector.scalar_tensor_tensor(
                        fa[:],
                        vc_sb[:, kvi, q0 : q0 + 512],
                        float(LAM),
                        o_ps[:],
                        op0=ALU.mult,
                        op1=ALU.add,
                    )
                    nc.gpsimd.tensor_mul(o_all[:, hd], fa[:], rs[:])

                def o_proj(qt, o_all):
                    q0 = qt * 512
                    for eo in range(D // P):
                        f_ps = psum2.tile([P, 512], F32, tag="sum")
                        for ei in range(H_Q):
                            nc.tensor.matmul(
                                f_ps[:],
                                wo_sb[:, ei, eo * P : (eo + 1) * P],
                                o_all[:, ei],
                                start=(ei == 0),
                                stop=(ei == H_Q - 1),
                            )
                        f_sb = fout.tile([P, 512], BF16, tag="fo")
                        nc.scalar.copy(f_sb[:], f_ps[:])
                        nc.sync.dma_start(
                            out_t.ap()[eo * P : (eo + 1) * P, q0 : q0 + 512],
                            f_sb[:],
                        )

                # interleave a tensor-heavy and a vector-heavy q-tile so the
                # small tiles' DVE chains hide under the big tiles' matmuls
                for qta, qtb in ((3, 0), (2, 1)):
                    oa = oall.tile([P, H_Q, 512], BF16, tag="oa", name="oa_a")
                    ob = oall.tile([P, H_Q, 512], BF16, tag="oa", name="oa_b")
                    for hd in range(H_Q):
                        attn_head(qta, hd, oa)
                        attn_head(qtb, hd, ob)
                    o_proj(qta, oa)
                    o_proj(qtb, ob)

    nc.compile()
    return nc


def _re3(a):
    """[K, E] -> [P, K//P, E] host rearrange for contiguous weight DMAs."""
    return np.ascontiguousarray(a.reshape(-1, P, a.shape[1]).transpose(1, 0, 2))


def _host_inputs(x, w_qkv, w_o):
    """Build the 8 per-core input maps from full inputs."""
    x = np.asarray(x, dtype=np.float32)
    w_qkv = np.asarray(w_qkv, dtype=np.float32)
    w_o = np.asarray(w_o, dtype=np.float32)

    half = HEAD_DIM // 2
    inv_freq = 1.0 / (
        THETA ** (np.arange(0, HEAD_DIM, 2, dtype=np.float32) / HEAD_DIM)
    )
    ang = np.arange(T, dtype=np.float32)[:, None] * inv_freq[None, :]  # [T, 64]
    cos = np.cos(ang).T.astype(np.float32)  # [64, T]
    sin = np.sin(ang).T.astype(np.float32)
    cos_t = np.ascontiguousarray(np.concatenate([cos, cos], axis=0)).astype(NPBF)
    sin_t = np.ascontiguousarray(np.concatenate([sin, sin], axis=0)).astype(NPBF)

    ones_m = np.ones((P, P), dtype=np.float32).astype(NPBF)
    pswap = np.zeros((P, P), dtype=np.float32)
    for p in range(half):
        pswap[p, p + half] = 1.0    # out[m=p+64] += ys[p]
        pswap[p + half, p] = -1.0   # out[m=p]    -= ys[p+64]
    pswap = pswap.astype(NPBF)
    ident = np.eye(P, dtype=np.float32).astype(NPBF)

    t_idx = np.arange(P, dtype=np.float32)[:, None]        # key within block
    j_idx = np.arange(512, dtype=np.float32)[None, :]      # query col
    mask_t = np.zeros((P, 2, 512), dtype=np.float32)
    for s_ in range(2):
        mask_t[:, s_] = 1.0 * (t_idx <= j_idx - 128 * s_)
    iota_t = np.broadcast_to(
        (np.arange(T, dtype=np.float32) + 1.0) * np.float32(LAM), (P, T)
    ).copy()

    in_maps = []
    for c in range(N_CORES):
        b, h = c // 2, c % 2
        qrows = slice(h * EQ, (h + 1) * EQ)
        krows = slice(Q_DIM + h * EKV, Q_DIM + (h + 1) * EKV)
        vrows = slice(Q_DIM + KV_DIM + h * EKV, Q_DIM + (h + 1) * EKV + KV_DIM)
        wq_r = _re3(np.ascontiguousarray(w_qkv[qrows].T * WSCALE))
        wq_r4 = np.ascontiguousarray(
            wq_r.reshape(P, KSUB, H_Q, P).transpose(2, 0, 1, 3)
        ).astype(NPF8)  # [H_Q, P, 16, 128]
        xt = np.ascontiguousarray(x[b].T)
        in_maps.append(
            {
                "x16t": xt.astype(NPBF),
                "x8t": xt.astype(NPF8),
                "wq8": wq_r4,
                "wk8": _re3(np.ascontiguousarray(w_qkv[krows].T * WSCALE)).astype(
                    NPF8
                ),
                "wv16": _re3(np.ascontiguousarray(w_qkv[vrows].T)).astype(NPBF),
                "wo16": _re3(
                    np.ascontiguousarray(w_o[:, h * EQ : (h + 1) * EQ].T)
                ).reshape(P, H_Q, D).astype(NPBF),
                "cos_t": cos_t,
                "sin_t": sin_t,
                "ones_m": ones_m,
                "pswap": pswap,
                "ident": ident,
                "mask_t": mask_t,
                "iota_t": iota_t,
            }
        )
    return in_maps


def _gather(results):
    out = np.empty((B, T, D), dtype=np.float32)
    for b in range(B):
        acc = np.asarray(results[2 * b]["out_t"], np.float32) + np.asarray(
            results[2 * b + 1]["out_t"], np.float32
        )
        out[b] = acc.T
    return out


_NC_CACHE = []


def _get_module():
    if not _NC_CACHE:
        _NC_CACHE.append(_build_module())
    return _NC_CACHE[0]


def kernel(x, w_qkv, w_o):
    nc = _get_module()
    in_maps = _host_inputs(x, w_qkv, w_o)
    results = bass2jax.run_bass_via_pjrt(nc, in_maps, n_cores=N_CORES)
    return _gather(results)


# revision 50
# speedup vs baseline: 1.1121x; 1.0415x over previous
"""Causal GQA attention block (QK L2-norm + RoPE) for 8 trn2 NeuronCores.

Sharding: tensor-parallel over head-halves (2) x data-parallel over batch (4).
Core c handles batch c//2 and heads [h*8, h*8+8) with h = c%2.

Fast-path design:
  - QK projection in fp8e4m3 DoubleRow (256-deep contraction per pass).
    Weights pre-scaled by 64; the L2 norm absorbs the scale exactly.
  - Linear softmax: with QK-norm the logits are bounded by +-0.0884, so
    exp(p) ~ 1 + p.  att = 1 + SCALE*s splits into an exact base (cumsum
    tables) plus an fp8 correction d8 (x) v8 run as DoubleRow matmuls.
  - Base numerator lam*cumsum(V^T) comes from a DVE prefix scan over
    PE-transposed V (no triangle matmuls); the denominator row-sum uses
    sum_k s_jk = q_j . cumsum(K)_j (one DVE mul + one ones-matmul per
    q-tile) instead of per-pair fp8 sum matmuls.
  - Diagonal score/AV matmuls run only over their causal column ranges.
  - Phase-1 norm/rope chain is pipelined across iterations: the ssq/swap
    matmuls of iteration i-1 are emitted after the raw matmuls of i, so
    the tensor queue never waits on the scalar/vector chain.
  - Phase-2 processes q-tiles in interleaved pairs (3,0) then (2,1) so
    the vector-heavy small tiles hide under the tensor-heavy large ones.
All scale factors are folded into host tables (lambda = 1/SCALE).
"""

import numpy as np
import ml_dtypes

import concourse.mybir as mybir
import concourse.tile as tile
from concourse import bacc
from concourse import bass2jax

F32 = mybir.dt.float32
F32R = mybir.dt.float32r
BF16 = mybir.dt.bfloat16
F8 = mybir.dt.float8e4
AF = mybir.ActivationFunctionType
ALU = mybir.AluOpType
PM = mybir.MatmulPerfMode

NPF8 = ml_dtypes.float8_e4m3
NPBF = ml_dtypes.bfloat16

P = 128
B, T, D = 4, 2048, 2048
N_HEADS, HEAD_DIM, N_KV = 16, 128, 4
Q_DIM = N_HEADS * HEAD_DIM          # 2048
KV_DIM = N_KV * HEAD_DIM            # 512
H_Q = 8                             # q heads per core
H_KV = 2                            # kv heads per core
EQ = H_Q * HEAD_DIM                 # 1024 q features per core
EKV = H_KV * HEAD_DIM               # 256
SCALE = 0.08838834764831845
LAM = 1.0 / SCALE
WSCALE = 64.0                       # fp8 pre-scale on w_qk (norm absorbs it)
THETA = 10000.0

KSUB = D // P                       # 16 contraction subtiles
KPAIR = KSUB // 2                   # 8 DoubleRow pairs
N_CORES = 8
TT_HALF = T // 2                    # 1024, phase-1 token half
NT512 = T // 512                    # 4 512-token q tiles
NTB = T // P                        # 16 128-token blocks


def _build_module():
    nc = bacc.Bacc("TRN2", target_bir_lowering=False, debug=False)

    x16t = nc.dram_tensor("x16t", [D, T], BF16, kind="ExternalInput")
    x8t = nc.dram_tensor("x8t", [D, T], F8, kind="ExternalInput")
    wq8 = nc.dram_tensor("wq8", [H_Q, P, KSUB, P], F8, kind="ExternalInput")
    wk8 = nc.dram_tensor("wk8", [P, KSUB, EKV], F8, kind="ExternalInput")
    wv16 = nc.dram_tensor("wv16", [P, KSUB, EKV], BF16, kind="ExternalInput")
    wo16 = nc.dram_tensor("wo16", [P, H_Q, D], BF16, kind="ExternalInput")
    cos_t = nc.dram_tensor("cos_t", [P, T], BF16, kind="ExternalInput")
    sin_t = nc.dram_tensor("sin_t", [P, T], BF16, kind="ExternalInput")
    ones_m = nc.dram_tensor("ones_m", [P, P], BF16, kind="ExternalInput")
    pswap = nc.dram_tensor("pswap", [P, P], BF16, kind="ExternalInput")
    ident = nc.dram_tensor("ident", [P, P], BF16, kind="ExternalInput")
    mask_t = nc.dram_tensor("mask_t", [P, 2, 512], F32R, kind="ExternalInput")
    iota_t = nc.dram_tensor("iota_t", [P, T], F32, kind="ExternalInput")
    out_t = nc.dram_tensor("out_t", [D, T], BF16, kind="ExternalOutput")

    with tile.TileContext(nc) as tc:
        with (
            tc.tile_pool(name="persist", bufs=1) as persist,
            tc.tile_pool(name="kv_persist", bufs=1) as kvp,
            tc.tile_pool(name="att_sb", bufs=8) as att_sb,
        ):
            ones_sb = persist.tile([P, P], BF16)
            psw_sb = persist.tile([P, P], BF16)
            id_sb = persist.tile([P, P], BF16)
            mask_sb = persist.tile([P, 2, 512], F32R)
            iota_sb = persist.tile([P, T], F32)
            nc.gpsimd.dma_start(ones_sb[:], ones_m.ap())
            nc.gpsimd.dma_start(psw_sb[:], pswap.ap())
            nc.gpsimd.dma_start(id_sb[:], ident.ap())
            nc.gpsimd.dma_start(mask_sb[:], mask_t.ap())
            nc.gpsimd.dma_start(iota_sb[:], iota_t.ap())
            k_sb = kvp.tile([P, H_KV, T], BF16)     # roped+normed K^T slabs
            v8_sb = kvp.tile([P, NTB, EKV], F8)     # V in [t, e] layout, fp8
            vT_sb = kvp.tile([P, H_KV, T], BF16)    # V^T in [e, t]
            q_all = kvp.tile([P, H_Q, T], BF16)     # Q resident in SBUF
            kc_sb = kvp.tile([P, H_KV, T], BF16)    # cumsum(K) along tokens
            vc_sb = kvp.tile([P, H_KV, T], F32)     # cumsum(V^T)

            # ---------------- phase 1: qkv proj + L2 norm + rope ----------
            with (
                tc.tile_pool(name="xres", bufs=1) as xres,
                tc.tile_pool(name="wstream", bufs=3) as wstream,
                tc.tile_pool(name="wvres", bufs=1) as wvres,
                tc.tile_pool(name="p1tmp", bufs=3) as p1tmp,
                tc.tile_pool(name="vstage", bufs=2) as vstage,
                tc.tile_pool(name="trig", bufs=1) as trig,
                tc.tile_pool(name="pp", bufs=4, space="PSUM") as pp,
                tc.tile_pool(name="pssq", bufs=2, space="PSUM") as pssq,
                tc.tile_pool(name="psw", bufs=2, space="PSUM") as psw,
            ):
                cos_sb = trig.tile([P, T], BF16)
                sin_sb = trig.tile([P, T], BF16)
                wv_sb = wvres.tile([P, KSUB, EKV], BF16)
                wk_sb = wvres.tile([P, KSUB, EKV], F8, name="wk_sb")
                nc.scalar.dma_start(wk_sb[:], wk8.ap())

                # deferred tail of the norm/rope chain: emitted one (es,tt)
                # iteration later so the tensor queue always has raw matmuls
                # in front of the chain-dependent ssq/swap matmuls
                pend = []

                def emit_tail(raw_ps, sq, dest, tg):
                    ssq_ps = pssq.tile([P, 512], F32, tag="ssq")
                    nc.tensor.matmul(
                        ssq_ps[:], ones_sb[:], sq[:], start=True, stop=True
                    )
                    s_sb = p1tmp.tile([P, 512], F32, tag="t2")
                    nc.scalar.activation(s_sb[:], ssq_ps[:], AF.Sqrt)
                    r_sb = p1tmp.tile([P, 512], F32, tag="t3")
                    nc.vector.reciprocal_approx_fast(r_sb[:], s_sb[:])
                    qn = p1tmp.tile([P, 512], BF16, tag="t4")
                    nc.vector.tensor_mul(qn[:], raw_ps[:], r_sb[:])
                    ys = p1tmp.tile([P, 512], BF16, tag="t5")
                    nc.vector.tensor_mul(ys[:], qn[:], sin_sb[:, tg : tg + 512])
                    sw_ps = psw.tile([P, 512], F32, tag="sw")
                    nc.tensor.matmul(
                        sw_ps[:], psw_sb[:], ys[:], start=True, stop=True
                    )
                    qc = p1tmp.tile([P, 512], BF16, tag="t6")
                    nc.gpsimd.tensor_mul(qc[:], qn[:], cos_sb[:, tg : tg + 512])
                    nc.vector.tensor_add(dest, sw_ps[:], qc[:])

                def flush():
                    while pend:
                        emit_tail(*pend.pop(0))

                for th in range(2):
                    t0 = th * TT_HALF
                    x8_sb = [
                        xres.tile(
                            [P, 2, TT_HALF], F8, tag=f"x8_{kp}", name=f"x8_{kp}"
                        )
                        for kp in range(KPAIR)
                    ]
                    x16_sb = [
                        xres.tile([P, TT_HALF], BF16, tag=f"x16_{ks}", name=f"x16_{ks}")
                        for ks in range(KSUB)
                    ]
                    xr16 = x16t.ap()[:, t0 : t0 + TT_HALF].rearrange(
                        "(ks p) t -> p ks t", p=P
                    )
                    xr8 = x8t.ap()[:, t0 : t0 + TT_HALF].rearrange(
                        "(ks p) t -> p ks t", p=P
                    )
                    # x8 first (pair 0 gates the first raw matmul); x16 on the
                    # gpsimd queue (only needed by the V projection).  One 2D
                    # DMA per ks slab (a fused [P,2,T/2] copy would be 3D).
                    for kp in range(KPAIR):
                        eng = nc.sync if kp % 2 == 0 else nc.scalar
                        for j in range(2):
                            eng.dma_start(x8_sb[kp][:, j], xr8[:, 2 * kp + j])
                    if th == 0:
                        nc.gpsimd.dma_start(cos_sb[:], cos_t.ap())
                        nc.gpsimd.dma_start(sin_sb[:], sin_t.ap())
                    for ks in range(KSUB):
                        nc.gpsimd.dma_start(x16_sb[ks][:], xr16[:, ks])
                    if th == 0:
                        nc.gpsimd.dma_start(wv_sb[:], wv16.ap())

                    def proj_norm_rope(es):
                        """project feature block es (fp8 DoubleRow), norm, rope"""
                        if es < H_Q:
                            w_sb = wstream.tile([P, KSUB, P], F8, tag="w")
                            nc.sync.dma_start(w_sb[:], wq8.ap()[es])
                            w_use = w_sb
                        else:
                            w_use = wk_sb
                        for tt in range(2):
                            tg = t0 + tt * 512
                            sl = slice(tt * 512, (tt + 1) * 512)
                            raw_ps = pp.tile([P, 512], F32, tag="raw")
                            for kp in range(KPAIR):
                                if es < H_Q:
                                    lhs = w_use[:, 2 * kp : 2 * kp + 2, :]
                                else:
                                    e0 = (es - H_Q) * P
                                    lhs = w_use[:, 2 * kp : 2 * kp + 2, e0 : e0 + P]
                                nc.tensor.matmul(
                                    raw_ps[:],
                                    lhs,
                                    x8_sb[kp][:, :, sl],
                                    start=(kp == 0),
                                    stop=(kp == KPAIR - 1),
                                    perf_mode=PM.DoubleRow,
                                )
                            sq = p1tmp.tile([P, 512], BF16, tag="t1")
                            nc.scalar.activation(sq[:], raw_ps[:], AF.Square)
                            if es < H_Q:
                                dest = q_all[:, es, tg : tg + 512]
                            else:
                                dest = k_sb[:, es - H_Q, tg : tg + 512]
                            if pend:
                                emit_tail(*pend.pop(0))
                            pend.append((raw_ps, sq, dest, tg))

                    # K first so downstream work can start earliest, then Q
                    for es in (H_Q, H_Q + 1):
                        proj_norm_rope(es)
                    if th == 1:
                        # K complete: start its prefix scan now so phase 2
                        # isn't gated on the end of the vector queue
                        flush()
                        for kvi in range(H_KV):
                            nc.vector.tensor_tensor_scan(
                                kc_sb[:, kvi],
                                k_sb[:, kvi],
                                k_sb[:, kvi],
                                0.0,
                                ALU.add,
                                ALU.bypass,
                            )
                    for es in range(H_Q):
                        proj_norm_rope(es)
                    flush()
                    # V projection + fp8 cast + PE transpose for the scan
                    for tb in range(TT_HALF // P):
                        tbg = th * (TT_HALF // P) + tb
                        v_ps = pp.tile([P, EKV], F32, tag="raw")
                        for ks in range(KSUB):
                            nc.tensor.matmul(
                                v_ps[:],
                                x16_sb[ks][:, tb * P : (tb + 1) * P],
                                wv_sb[:, ks],
                                start=(ks == 0),
                                stop=(ks == KSUB - 1),
                            )
                        vst = vstage.tile([P, EKV], BF16, tag="vs")
                        nc.scalar.copy(vst[:], v_ps[:])
                        nc.scalar.copy(v8_sb[:, tbg], v_ps[:])
                        for kvi in range(H_KV):
                            tp_ps = pssq.tile([P, P], BF16, tag="ssq")
                            nc.tensor.transpose(
                                tp_ps[:],
                                vst[:, kvi * P : (kvi + 1) * P],
                                id_sb[:],
                            )
                            nc.scalar.copy(
                                vT_sb[:, kvi, tbg * P : (tbg + 1) * P], tp_ps[:]
                            )
                # V^T complete: full-length prefix scan
                for kvi in range(H_KV):
                    nc.vector.tensor_tensor_scan(
                        vc_sb[:, kvi],
                        vT_sb[:, kvi],
                        vT_sb[:, kvi],
                        0.0,
                        ALU.add,
                        ALU.bypass,
                    )

            # ------- phase 2: attention + output projection per q-tile ----
            with (
                tc.tile_pool(name="wores", bufs=1) as wores,
                tc.tile_pool(name="p2tmp", bufs=4) as p2tmp,
                tc.tile_pool(name="oall", bufs=2) as oall,
                tc.tile_pool(name="fout", bufs=3) as fout,
                tc.tile_pool(name="psc", bufs=4, space="PSUM") as psc,
                tc.tile_pool(name="pav", bufs=2, space="PSUM") as pav,
                tc.tile_pool(name="psum2", bufs=2, space="PSUM") as psum2,
            ):
                wo_sb = wores.tile([P, H_Q, D], BF16)
                for ei in range(H_Q):
                    nc.gpsimd.dma_start(wo_sb[:, ei], wo16.ap()[:, ei])

                def attn_head(qt, hd, o_all):
                    q0 = qt * 512
                    nkb = (qt + 1) * 4
                    npair = nkb // 2
                    kvi = hd // 4
                    o_ps = pav.tile([P, 512], F32, tag="av")

                    # denominator numerand: q . Kc (exact causal row-sum)
                    qkc = p2tmp.tile([P, 512], BF16, tag="qk")
                    nc.gpsimd.tensor_mul(
                        qkc[:],
                        q_all[:, hd, q0 : q0 + 512],
                        kc_sb[:, kvi, q0 : q0 + 512],
                    )
                    # all scores + casts first (per-block psum tiles, 4-deep
                    # rotation hides cast latency), then all AV matmuls
                    d8s = []
                    for pj in range(npair - 2):
                        d8 = att_sb.tile([P, 2, 512], F8, tag="att")
                        for j in range(2):
                            kb = 2 * pj + j
                            sc_ps = psc.tile([P, 512], F32, tag="sc")
                            nc.tensor.matmul(
                                sc_ps[:],
                                k_sb[:, kvi, kb * P : (kb + 1) * P],
                                q_all[:, hd, q0 : q0 + 512],
                                start=True,
                                stop=True,
                            )
                            nc.scalar.activation(d8[:, j], sc_ps[:], AF.Copy)
                        d8s.append(d8)
                    # diagonal pair A (rel blocks 0,1): masked, full width
                    d8a = att_sb.tile([P, 2, 512], F8, tag="att")
                    for j in range(2):
                        sc_ps = psc.tile([P, 512], F32, tag="sc")
                        nc.tensor.matmul(
                            sc_ps[:],
                            k_sb[:, kvi, (nkb - 4 + j) * P : (nkb - 3 + j) * P],
                            q_all[:, hd, q0 : q0 + 512],
                            start=True,
                            stop=True,
                        )
                        nc.vector.tensor_mul(d8a[:, j], sc_ps[:], mask_sb[:, j])
                    # diagonal pair B (rel blocks 2,3): cols [256:512)
                    d8b = att_sb.tile([P, 2, 256], F8, tag="attb")
                    for j in range(2):
                        sc_ps = psc.tile([P, 256], F32, tag="sc")
                        nc.tensor.matmul(
                            sc_ps[:],
                            k_sb[:, kvi, (nkb - 2 + j) * P : (nkb - 1 + j) * P],
                            q_all[:, hd, q0 + 256 : q0 + 512],
                            start=True,
                            stop=True,
                        )
                        nc.vector.tensor_mul(
                            d8b[:, j], sc_ps[:], mask_sb[:, j, :256]
                        )
                    for pj in range(npair - 2):
                        kb0 = 2 * pj
                        nc.tensor.matmul(
                            o_ps[:],
                            v8_sb[:, kb0 : kb0 + 2, kvi * P : (kvi + 1) * P],
                            d8s[pj][:],
                            start=(pj == 0),
                            stop=False,
                            perf_mode=PM.DoubleRow,
                        )
                    nc.tensor.matmul(
                        o_ps[:],
                        v8_sb[:, nkb - 4 : nkb - 2, kvi * P : (kvi + 1) * P],
                        d8a[:],
                        start=(npair == 2),
                        stop=True,
                        perf_mode=PM.DoubleRow,
                    )
                    nc.tensor.matmul(
                        o_ps[:, 256:],
                        v8_sb[:, nkb - 2 : nkb, kvi * P : (kvi + 1) * P],
                        d8b[:],
                        start=False,
                        stop=True,
                        perf_mode=PM.DoubleRow,
                        skip_group_check=True,
                    )
                    # denominator: iota + sum(q . Kc)
                    den_ps = psum2.tile([P, 512], F32, tag="sum")
                    nc.tensor.matmul(
                        den_ps[:], ones_sb[:], qkc[:], start=True, stop=True
                    )
                    den = p2tmp.tile([P, 512], F32, tag="dn")
                    nc.vector.tensor_add(
                        den[:], den_ps[:], iota_sb[:, q0 : q0 + 512]
                    )
                    rs = p2tmp.tile([P, 512], F32, tag="rs")
                    nc.vector.reciprocal_approx_fast(rs[:], den[:])
                    # finalize: (corr + lam*Vc) / den
                    fa = p2tmp.tile([P, 512], BF16, tag="fa")
                    nc.vector.scalar_tensor_tensor(
                        fa[:],
                        vc_sb[:, kvi, q0 : q0 + 512],
                        float(LAM),
                        o_ps[:],
                        op0=ALU.mult,
                        op1=ALU.add,
                    )
                    nc.gpsimd.tensor_mul(o_all[:, hd], fa[:], rs[:])

                def o_proj(qt, o_all):
                    q0 = qt * 512
                    for eo in range(D // P):
                        f_ps = psum2.tile([P, 512], F32, tag="sum")
                        for ei in range(H_Q):
                            nc.tensor.matmul(
                                f_ps[:],
                                wo_sb[:, ei, eo * P : (eo + 1) * P],
                                o_all[:, ei],
                                start=(ei == 0),
                                stop=(ei == H_Q - 1),
                            )
                        f_sb = fout.tile([P, 512], BF16, tag="fo")
                        nc.scalar.copy(f_sb[:], f_ps[:])
                        nc.sync.dma_start(
                            out_t.ap()[eo * P : (eo + 1) * P, q0 : q0 + 512],
                            f_sb[:],
                        )

                # interleave a tensor-heavy and a vector-heavy q-tile so the
                # small tiles' DVE chains hide under the big tiles' matmuls
                for qta, qtb in ((3, 0), (2, 1)):
                    oa = oall.tile([P, H_Q, 512], BF16, tag="oa", name="oa_a")
                    ob = oall.tile([P, H_Q, 512], BF16, tag="oa", name="oa_b")
                    for hd in range(H_Q):
                        attn_head(qta, hd, oa)
                        attn_head(qtb, hd, ob)
                    o_proj(qta, oa)
                    o_proj(qtb, ob)

    nc.compile()
    return nc


def _re3(a):
    """[K, E] -> [P, K//P, E] host rearrange for contiguous weight DMAs."""
    return np.ascontiguousarray(a.reshape(-1, P, a.shape[1]).transpose(1, 0, 2))


def _host_inputs(x, w_qkv, w_o):
    """Build the 8 per-core input maps from full inputs."""
    x = np.asarray(x, dtype=np.float32)
    w_qkv = np.asarray(w_qkv, dtype=np.float32)
    w_o = np.asarray(w_o, dtype=np.float32)

    half = HEAD_DIM // 2
    inv_freq = 1.0 / (
        THETA ** (np.arange(0, HEAD_DIM, 2, dtype=np.float32) / HEAD_DIM)
    )
    ang = np.arange(T, dtype=np.float32)[:, None] * inv_freq[None, :]  # [T, 64]
    cos = np.cos(ang).T.astype(np.float32)  # [64, T]
    sin = np.sin(ang).T.astype(np.float32)
    cos_t = np.ascontiguousarray(np.concatenate([cos, cos], axis=0)).astype(NPBF)
    sin_t = np.ascontiguousarray(np.concatenate([sin, sin], axis=0)).astype(NPBF)

    ones_m = np.ones((P, P), dtype=np.float32).astype(NPBF)
    pswap = np.zeros((P, P), dtype=np.float32)
    for p in range(half):
        pswap[p, p + half] = 1.0    # out[m=p+64] += ys[p]
        pswap[p + half, p] = -1.0   # out[m=p]    -= ys[p+64]
    pswap = pswap.astype(NPBF)
    ident = np.eye(P, dtype=np.float32).astype(NPBF)

    t_idx = np.arange(P, dtype=np.float32)[:, None]        # key within block
    j_idx = np.arange(512, dtype=np.float32)[None, :]      # query col
    mask_t = np.zeros((P, 2, 512), dtype=np.float32)
    for s_ in range(2):
        mask_t[:, s_] = 1.0 * (t_idx <= j_idx - 128 * s_)
    iota_t = np.broadcast_to(
        (np.arange(T, dtype=np.float32) + 1.0) * np.float32(LAM), (P, T)
    ).copy()

    in_maps = []
    for c in range(N_CORES):
        b, h = c // 2, c % 2
        qrows = slice(h * EQ, (h + 1) * EQ)
        krows = slice(Q_DIM + h * EKV, Q_DIM + (h + 1) * EKV)
        vrows = slice(Q_DIM + KV_DIM + h * EKV, Q_DIM + (h + 1) * EKV + KV_DIM)
        wq_r = _re3(np.ascontiguousarray(w_qkv[qrows].T * WSCALE))
        wq_r4 = np.ascontiguousarray(
            wq_r.reshape(P, KSUB, H_Q, P).transpose(2, 0, 1, 3)
        ).astype(NPF8)  # [H_Q, P, 16, 128]
        xt = np.ascontiguousarray(x[b].T)
        in_maps.append(
            {
                "x16t": xt.astype(NPBF),
                "x8t": xt.astype(NPF8),
                "wq8": wq_r4,
                "wk8": _re3(np.ascontiguousarray(w_qkv[krows].T * WSCALE)).astype(
                    NPF8
                ),
                "wv16": _re3(np.ascontiguousarray(w_qkv[vrows].T)).astype(NPBF),
                "wo16": _re3(
                    np.ascontiguousarray(w_o[:, h * EQ : (h + 1) * EQ].T)
                ).reshape(P, H_Q, D).astype(NPBF),
                "cos_t": cos_t,
                "sin_t": sin_t,
                "ones_m": ones_m,
                "pswap": pswap,
                "ident": ident,
                "mask_t": mask_t,
                "iota_t": iota_t,
            }
        )
    return in_maps


def _gather(results):
    out = np.empty((B, T, D), dtype=np.float32)
    for b in range(B):
        acc = np.asarray(results[2 * b]["out_t"], np.float32) + np.asarray(
            results[2 * b + 1]["out_t"], np.float32
        )
        out[b] = acc.T
    return out


_NC_CACHE = []


def _get_module():
    if not _NC_CACHE:
        _NC_CACHE.append(_build_module())
    return _NC_CACHE[0]


def kernel(x, w_qkv, w_o):
    nc = _get_module()
    in_maps = _host_inputs(x, w_qkv, w_o)
    results = bass2jax.run_bass_via_pjrt(nc, in_maps, n_cores=N_CORES)
    return _gather(results)


# revision 53
# speedup vs baseline: 1.1154x; 1.0030x over previous
"""Causal GQA attention block (QK L2-norm + RoPE) for 8 trn2 NeuronCores.

Sharding: tensor-parallel over head-halves (2) x data-parallel over batch (4).
Core c handles batch c//2 and heads [h*8, h*8+8) with h = c%2.

Fast-path design:
  - QK projection in fp8e4m3 DoubleRow (256-deep contraction per pass).
    Weights pre-scaled by 64; the L2 norm absorbs the scale exactly.
  - Linear softmax: with QK-norm the logits are bounded by +-0.0884, so
    exp(p) ~ 1 + p.  att = 1 + SCALE*s splits into an exact base (cumsum
    tables) plus an fp8 correction d8 (x) v8 run as DoubleRow matmuls.
  - Base numerator lam*cumsum(V^T) comes from a DVE prefix scan over
    PE-transposed V (no triangle matmuls); the denominator row-sum uses
    sum_k s_jk = q_j . cumsum(K)_j (one DVE mul + one ones-matmul per
    q-tile) instead of per-pair fp8 sum matmuls.
  - Diagonal score/AV matmuls run only over their causal column ranges.
  - Phase-1 norm/rope chain is pipelined across iterations: the ssq/swap
    matmuls of iteration i-1 are emitted after the raw matmuls of i, so
    the tensor queue never waits on the scalar/vector chain.
  - Phase-2 processes q-tiles in interleaved pairs (3,0) then (2,1) so
    the vector-heavy small tiles hide under the tensor-heavy large ones.
All scale factors are folded into host tables (lambda = 1/SCALE).
"""

import numpy as np
import ml_dtypes

import concourse.mybir as mybir
import concourse.tile as tile
from concourse import bacc
from concourse import bass2jax

F32 = mybir.dt.float32
F32R = mybir.dt.float32r
BF16 = mybir.dt.bfloat16
F8 = mybir.dt.float8e4
AF = mybir.ActivationFunctionType
ALU = mybir.AluOpType
PM = mybir.MatmulPerfMode

NPF8 = ml_dtypes.float8_e4m3
NPBF = ml_dtypes.bfloat16

P = 128
B, T, D = 4, 2048, 2048
N_HEADS, HEAD_DIM, N_KV = 16, 128, 4
Q_DIM = N_HEADS * HEAD_DIM          # 2048
KV_DIM = N_KV * HEAD_DIM            # 512
H_Q = 8                             # q heads per core
H_KV = 2                            # kv heads per core
EQ = H_Q * HEAD_DIM                 # 1024 q features per core
EKV = H_KV * HEAD_DIM               # 256
SCALE = 0.08838834764831845
LAM = 1.0 / SCALE
WSCALE = 64.0                       # fp8 pre-scale on w_qk (norm absorbs it)
THETA = 10000.0

KSUB = D // P                       # 16 contraction subtiles
KPAIR = KSUB // 2                   # 8 DoubleRow pairs
N_CORES = 8
TT_HALF = T // 2                    # 1024, phase-1 token half
NT512 = T // 512                    # 4 512-token q tiles
NTB = T // P                        # 16 128-token blocks


def _build_module():
    nc = bacc.Bacc("TRN2", target_bir_lowering=False, debug=False)

    x16t = nc.dram_tensor("x16t", [D, T], BF16, kind="ExternalInput")
    x8t = nc.dram_tensor("x8t", [D, T], F8, kind="ExternalInput")
    wq8 = nc.dram_tensor("wq8", [H_Q, P, KSUB, P], F8, kind="ExternalInput")
    wk8 = nc.dram_tensor("wk8", [P, KSUB, EKV], F8, kind="ExternalInput")
    wv16 = nc.dram_tensor("wv16", [P, KSUB, EKV], BF16, kind="ExternalInput")
    wo16 = nc.dram_tensor("wo16", [P, H_Q, D], BF16, kind="ExternalInput")
    cos_t = nc.dram_tensor("cos_t", [P, T], BF16, kind="ExternalInput")
    sin_t = nc.dram_tensor("sin_t", [P, T], BF16, kind="ExternalInput")
    ones_m = nc.dram_tensor("ones_m", [P, P], BF16, kind="ExternalInput")
    pswap = nc.dram_tensor("pswap", [P, P], BF16, kind="ExternalInput")
    ident = nc.dram_tensor("ident", [P, P], BF16, kind="ExternalInput")
    mask_t = nc.dram_tensor("mask_t", [P, 2, 512], F32R, kind="ExternalInput")
    iota_t = nc.dram_tensor("iota_t", [P, T], F32, kind="ExternalInput")
    out_t = nc.dram_tensor("out_t", [D, T], BF16, kind="ExternalOutput")

    with tile.TileContext(nc) as tc:
        with (
            tc.tile_pool(name="persist", bufs=1) as persist,
            tc.tile_pool(name="kv_persist", bufs=1) as kvp,
            tc.tile_pool(name="att_sb", bufs=8) as att_sb,
        ):
            ones_sb = persist.tile([P, P], BF16)
            psw_sb = persist.tile([P, P], BF16)
            id_sb = persist.tile([P, P], BF16)
            mask_sb = persist.tile([P, 2, 512], F32R)
            iota_sb = persist.tile([P, T], F32)
            nc.gpsimd.dma_start(ones_sb[:], ones_m.ap())
            nc.gpsimd.dma_start(psw_sb[:], pswap.ap())
            nc.gpsimd.dma_start(id_sb[:], ident.ap())
            nc.gpsimd.dma_start(mask_sb[:], mask_t.ap())
            nc.gpsimd.dma_start(iota_sb[:], iota_t.ap())
            k_sb = kvp.tile([P, H_KV, T], BF16)     # roped+normed K^T slabs
            v8_sb = kvp.tile([P, NTB, EKV], F8)     # V in [t, e] layout, fp8
            vT_sb = kvp.tile([P, H_KV, T], BF16)    # V^T in [e, t]
            q_all = kvp.tile([P, H_Q, T], BF16)     # Q resident in SBUF
            kc_sb = kvp.tile([P, H_KV, T], BF16)    # cumsum(K) along tokens
            vc_sb = kvp.tile([P, H_KV, T], F32)     # cumsum(V^T)

            # ---------------- phase 1: qkv proj + L2 norm + rope ----------
            with (
                tc.tile_pool(name="xres", bufs=1) as xres,
                tc.tile_pool(name="wstream", bufs=3) as wstream,
                tc.tile_pool(name="wvres", bufs=1) as wvres,
                tc.tile_pool(name="p1tmp", bufs=3) as p1tmp,
                tc.tile_pool(name="vstage", bufs=2) as vstage,
                tc.tile_pool(name="trig", bufs=1) as trig,
                tc.tile_pool(name="pp", bufs=4, space="PSUM") as pp,
                tc.tile_pool(name="pssq", bufs=2, space="PSUM") as pssq,
                tc.tile_pool(name="psw", bufs=2, space="PSUM") as psw,
            ):
                cos_sb = trig.tile([P, T], BF16)
                sin_sb = trig.tile([P, T], BF16)
                wv_sb = wvres.tile([P, KSUB, EKV], BF16)
                wk_sb = wvres.tile([P, KSUB, EKV], F8, name="wk_sb")
                nc.scalar.dma_start(wk_sb[:], wk8.ap())

                # two-stage deferred tail of the norm/rope chain: the norm
                # DVE chain runs one iteration behind the raw matmuls, the
                # swap matmul + rope combine three behind, so the tensor
                # queue never waits on a same-iteration cross-engine result
                p1q = []
                p2q = []

                def emit_part1(c):
                    raw_ps, sq, dest, tg = c
                    ssq_ps = pssq.tile([P, 512], F32, tag="ssq")
                    nc.tensor.matmul(
                        ssq_ps[:], ones_sb[:], sq[:], start=True, stop=True
                    )
                    s_sb = p1tmp.tile([P, 512], F32, tag="t2")
                    nc.scalar.activation(s_sb[:], ssq_ps[:], AF.Sqrt)
                    r_sb = p1tmp.tile([P, 512], F32, tag="t3")
                    nc.vector.reciprocal_approx_fast(r_sb[:], s_sb[:])
                    qn = p1tmp.tile([P, 512], BF16, tag="t4")
                    nc.vector.tensor_mul(qn[:], raw_ps[:], r_sb[:])
                    ys = p1tmp.tile([P, 512], BF16, tag="t5")
                    nc.vector.tensor_mul(ys[:], qn[:], sin_sb[:, tg : tg + 512])
                    qc = p1tmp.tile([P, 512], BF16, tag="t6")
                    nc.gpsimd.tensor_mul(qc[:], qn[:], cos_sb[:, tg : tg + 512])
                    return (ys, qc, dest)

                def emit_part2(c):
                    ys, qc, dest = c
                    sw_ps = psw.tile([P, 512], F32, tag="sw")
                    nc.tensor.matmul(
                        sw_ps[:], psw_sb[:], ys[:], start=True, stop=True
                    )
                    nc.vector.tensor_add(dest, sw_ps[:], qc[:])

                def step():
                    if len(p1q) > 1:
                        p2q.append(emit_part1(p1q.pop(0)))
                    if len(p2q) > 2:
                        emit_part2(p2q.pop(0))

                def flush():
                    while p1q:
                        p2q.append(emit_part1(p1q.pop(0)))
                    while p2q:
                        emit_part2(p2q.pop(0))

                for th in range(2):
                    t0 = th * TT_HALF
                    x8_sb = [
                        xres.tile(
                            [P, 2, TT_HALF], F8, tag=f"x8_{kp}", name=f"x8_{kp}"
                        )
                        for kp in range(KPAIR)
                    ]
                    x16_sb = [
                        xres.tile([P, TT_HALF], BF16, tag=f"x16_{ks}", name=f"x16_{ks}")
                        for ks in range(KSUB)
                    ]
                    xr16 = x16t.ap()[:, t0 : t0 + TT_HALF].rearrange(
                        "(ks p) t -> p ks t", p=P
                    )
                    xr8 = x8t.ap()[:, t0 : t0 + TT_HALF].rearrange(
                        "(ks p) t -> p ks t", p=P
                    )
                    # x8 first (pair 0 gates the first raw matmul); x16 on the
                    # gpsimd queue (only needed by the V projection).  One 2D
                    # DMA per ks slab (a fused [P,2,T/2] copy would be 3D).
                    for kp in range(KPAIR):
                        eng = nc.sync if kp % 2 == 0 else nc.scalar
                        for j in range(2):
                            eng.dma_start(x8_sb[kp][:, j], xr8[:, 2 * kp + j])
                    if th == 0:
                        nc.gpsimd.dma_start(cos_sb[:], cos_t.ap())
                        nc.gpsimd.dma_start(sin_sb[:], sin_t.ap())
                    for ks in range(KSUB):
                        nc.gpsimd.dma_start(x16_sb[ks][:], xr16[:, ks])
                    if th == 0:
                        nc.gpsimd.dma_start(wv_sb[:], wv16.ap())

                    def proj_norm_rope(es):
                        """project feature block es (fp8 DoubleRow), norm, rope"""
                        if es < H_Q:
                            w_sb = wstream.tile([P, KSUB, P], F8, tag="w")
                            nc.sync.dma_start(w_sb[:], wq8.ap()[es])
                            w_use = w_sb
                        else:
                            w_use = wk_sb
                        for tt in range(2):
                            tg = t0 + tt * 512
                            sl = slice(tt * 512, (tt + 1) * 512)
                            raw_ps = pp.tile([P, 512], F32, tag="raw")
                            for kp in range(KPAIR):
                                if es < H_Q:
                                    lhs = w_use[:, 2 * kp : 2 * kp + 2, :]
                                else:
                                    e0 = (es - H_Q) * P
                                    lhs = w_use[:, 2 * kp : 2 * kp + 2, e0 : e0 + P]
                                nc.tensor.matmul(
                                    raw_ps[:],
                                    lhs,
                                    x8_sb[kp][:, :, sl],
                                    start=(kp == 0),
                                    stop=(kp == KPAIR - 1),
                                    perf_mode=PM.DoubleRow,
                                )
                            sq = p1tmp.tile([P, 512], BF16, tag="t1")
                            nc.scalar.activation(sq[:], raw_ps[:], AF.Square)
                            if es < H_Q:
                                dest = q_all[:, es, tg : tg + 512]
                            else:
                                dest = k_sb[:, es - H_Q, tg : tg + 512]
                            p1q.append((raw_ps, sq, dest, tg))
                            step()

                    # K first so downstream work can start earliest, then Q
                    for es in (H_Q, H_Q + 1):
                        proj_norm_rope(es)
                    if th == 1:
                        # K complete: start its prefix scan now so phase 2
                        # isn't gated on the end of the vector queue
                        flush()
                        for kvi in range(H_KV):
                            nc.vector.tensor_tensor_scan(
                                kc_sb[:, kvi],
                                k_sb[:, kvi],
                                k_sb[:, kvi],
                                0.0,
                                ALU.add,
                                ALU.bypass,
                            )
                    for es in range(H_Q):
                        proj_norm_rope(es)
                    flush()
                    # V projection + fp8 cast + PE transpose for the scan
                    for tb in range(TT_HALF // P):
                        tbg = th * (TT_HALF // P) + tb
                        v_ps = pp.tile([P, EKV], F32, tag="raw")
                        for ks in range(KSUB):
                            nc.tensor.matmul(
                                v_ps[:],
                                x16_sb[ks][:, tb * P : (tb + 1) * P],
                                wv_sb[:, ks],
                                start=(ks == 0),
                                stop=(ks == KSUB - 1),
                            )
                        vst = vstage.tile([P, EKV], BF16, tag="vs")
                        nc.scalar.copy(vst[:], v_ps[:])
                        nc.scalar.copy(v8_sb[:, tbg], v_ps[:])
                        for kvi in range(H_KV):
                            tp_ps = pssq.tile([P, P], BF16, tag="ssq")
                            nc.tensor.transpose(
                                tp_ps[:],
                                vst[:, kvi * P : (kvi + 1) * P],
                                id_sb[:],
                            )
                            nc.scalar.copy(
                                vT_sb[:, kvi, tbg * P : (tbg + 1) * P], tp_ps[:]
                            )
                # V^T complete: full-length prefix scan
                for kvi in range(H_KV):
                    nc.vector.tensor_tensor_scan(
                        vc_sb[:, kvi],
                        vT_sb[:, kvi],
                        vT_sb[:, kvi],
                        0.0,
                        ALU.add,
                        ALU.bypass,
                    )

            # ------- phase 2: attention + output projection per q-tile ----
            with (
                tc.tile_pool(name="wores", bufs=1) as wores,
                tc.tile_pool(name="p2tmp", bufs=4) as p2tmp,
                tc.tile_pool(name="oall", bufs=2) as oall,
                tc.tile_pool(name="fout", bufs=3) as fout,
                tc.tile_pool(name="psc", bufs=4, space="PSUM") as psc,
                tc.tile_pool(name="pav", bufs=2, space="PSUM") as pav,
                tc.tile_pool(name="psum2", bufs=2, space="PSUM") as psum2,
            ):
                wo_sb = wores.tile([P, H_Q, D], BF16)
                for ei in range(H_Q):
                    nc.gpsimd.dma_start(wo_sb[:, ei], wo16.ap()[:, ei])

                def attn_head(qt, hd, o_all):
                    q0 = qt * 512
                    nkb = (qt + 1) * 4
                    npair = nkb // 2
                    kvi = hd // 4
                    o_ps = pav.tile([P, 512], F32, tag="av")

                    # denominator numerand: q . Kc (exact causal row-sum)
                    qkc = p2tmp.tile([P, 512], BF16, tag="qk")
                    nc.gpsimd.tensor_mul(
                        qkc[:],
                        q_all[:, hd, q0 : q0 + 512],
                        kc_sb[:, kvi, q0 : q0 + 512],
                    )
                    # all scores + casts first (per-block psum tiles, 4-deep
                    # rotation hides cast latency), then all AV matmuls
                    d8s = []
                    for pj in range(npair - 2):
                        d8 = att_sb.tile([P, 2, 512], F8, tag="att")
                        for j in range(2):
                            kb = 2 * pj + j
                            sc_ps = psc.tile([P, 512], F32, tag="sc")
                            nc.tensor.matmul(
                                sc_ps[:],
                                k_sb[:, kvi, kb * P : (kb + 1) * P],
                                q_all[:, hd, q0 : q0 + 512],
                                start=True,
                                stop=True,
                            )
                            nc.scalar.activation(d8[:, j], sc_ps[:], AF.Copy)
                        d8s.append(d8)
                    # diagonal pair A (rel blocks 0,1): masked; block 1 only
                    # needs cols >= 128 (the masked-off psum region holds
                    # stale-but-finite scores from earlier full writes)
                    d8a = att_sb.tile([P, 2, 512], F8, tag="att")
                    for j in range(2):
                        sc_ps = psc.tile([P, 512], F32, tag="sc")
                        nc.tensor.matmul(
                            sc_ps[:, 128 * j :],
                            k_sb[:, kvi, (nkb - 4 + j) * P : (nkb - 3 + j) * P],
                            q_all[:, hd, q0 + 128 * j : q0 + 512],
                            start=True,
                            stop=True,
                        )
                        nc.vector.tensor_mul(d8a[:, j], sc_ps[:], mask_sb[:, j])
                    # diagonal pair B (rel blocks 2,3): cols [256:512)
                    d8b = att_sb.tile([P, 2, 256], F8, tag="attb")
                    for j in range(2):
                        sc_ps = psc.tile([P, 256], F32, tag="sc")
                        nc.tensor.matmul(
                            sc_ps[:, 128 * j :],
                            k_sb[:, kvi, (nkb - 2 + j) * P : (nkb - 1 + j) * P],
                            q_all[:, hd, q0 + 256 + 128 * j : q0 + 512],
                            start=True,
                            stop=True,
                        )
                        nc.vector.tensor_mul(
                            d8b[:, j], sc_ps[:], mask_sb[:, j, :256]
                        )
                    for pj in range(npair - 2):
                        kb0 = 2 * pj
                        nc.tensor.matmul(
                            o_ps[:],
                            v8_sb[:, kb0 : kb0 + 2, kvi * P : (kvi + 1) * P],
                            d8s[pj][:],
                            start=(pj == 0),
                            stop=False,
                            perf_mode=PM.DoubleRow,
                        )
                    nc.tensor.matmul(
                        o_ps[:],
                        v8_sb[:, nkb - 4 : nkb - 2, kvi * P : (kvi + 1) * P],
                        d8a[:],
                        start=(npair == 2),
                        stop=True,
                        perf_mode=PM.DoubleRow,
                    )
                    nc.tensor.matmul(
                        o_ps[:, 256:],
                        v8_sb[:, nkb - 2 : nkb, kvi * P : (kvi + 1) * P],
                        d8b[:],
                        start=False,
                        stop=True,
                        perf_mode=PM.DoubleRow,
                        skip_group_check=True,
                    )
                    # denominator: iota + sum(q . Kc)
                    den_ps = psum2.tile([P, 512], F32, tag="sum")
                    nc.tensor.matmul(
                        den_ps[:], ones_sb[:], qkc[:], start=True, stop=True
                    )
                    den = p2tmp.tile([P, 512], F32, tag="dn")
                    nc.vector.tensor_add(
                        den[:], den_ps[:], iota_sb[:, q0 : q0 + 512]
                    )
                    rs = p2tmp.tile([P, 512], F32, tag="rs")
                    nc.vector.reciprocal_approx_fast(rs[:], den[:])
                    # finalize: (corr + lam*Vc) / den
                    fa = p2tmp.tile([P, 512], BF16, tag="fa")
                    nc.vector.scalar_tensor_tensor(
                        fa[:],
                        vc_sb[:, kvi, q0 : q0 + 512],
                        float(LAM),
                        o_ps[:],
                        op0=ALU.mult,
                        op1=ALU.add,
                    )
                    nc.gpsimd.tensor_mul(o_all[:, hd], fa[:], rs[:])

                def o_proj(qt, o_all):
                    q0 = qt * 512
                    for eo in range(D // P):
                        f_ps = psum2.tile([P, 512], F32, tag="sum")
                        for ei in range(H_Q):
                            nc.tensor.matmul(
                                f_ps[:],
                                wo_sb[:, ei, eo * P : (eo + 1) * P],
                                o_all[:, ei],
                                start=(ei == 0),
                                stop=(ei == H_Q - 1),
                            )
                        f_sb = fout.tile([P, 512], BF16, tag="fo")
                        nc.scalar.copy(f_sb[:], f_ps[:])
                        nc.sync.dma_start(
                            out_t.ap()[eo * P : (eo + 1) * P, q0 : q0 + 512],
                            f_sb[:],
                        )

                # interleave a tensor-heavy and a vector-heavy q-tile so the
                # small tiles' DVE chains hide under the big tiles' matmuls
                for qta, qtb in ((3, 0), (2, 1)):
                    oa = oall.tile([P, H_Q, 512], BF16, tag="oa", name="oa_a")
                    ob = oall.tile([P, H_Q, 512], BF16, tag="oa", name="oa_b")
                    for hd in range(H_Q):
                        attn_head(qta, hd, oa)
                        attn_head(qtb, hd, ob)
                    o_proj(qta, oa)
                    o_proj(qtb, ob)

    nc.compile()
    return nc


def _re3(a):
    """[K, E] -> [P, K//P, E] host rearrange for contiguous weight DMAs."""
    return np.ascontiguousarray(a.reshape(-1, P, a.shape[1]).transpose(1, 0, 2))


def _host_inputs(x, w_qkv, w_o):
    """Build the 8 per-core input maps from full inputs."""
    x = np.asarray(x, dtype=np.float32)
    w_qkv = np.asarray(w_qkv, dtype=np.float32)
    w_o = np.asarray(w_o, dtype=np.float32)

    half = HEAD_DIM // 2
    inv_freq = 1.0 / (
        THETA ** (np.arange(0, HEAD_DIM, 2, dtype=np.float32) / HEAD_DIM)
    )
    ang = np.arange(T, dtype=np.float32)[:, None] * inv_freq[None, :]  # [T, 64]
    cos = np.cos(ang).T.astype(np.float32)  # [64, T]
    sin = np.sin(ang).T.astype(np.float32)
    cos_t = np.ascontiguousarray(np.concatenate([cos, cos], axis=0)).astype(NPBF)
    sin_t = np.ascontiguousarray(np.concatenate([sin, sin], axis=0)).astype(NPBF)

    ones_m = np.ones((P, P), dtype=np.float32).astype(NPBF)
    pswap = np.zeros((P, P), dtype=np.float32)
    for p in range(half):
        pswap[p, p + half] = 1.0    # out[m=p+64] += ys[p]
        pswap[p + half, p] = -1.0   # out[m=p]    -= ys[p+64]
    pswap = pswap.astype(NPBF)
    ident = np.eye(P, dtype=np.float32).astype(NPBF)

    t_idx = np.arange(P, dtype=np.float32)[:, None]        # key within block
    j_idx = np.arange(512, dtype=np.float32)[None, :]      # query col
    mask_t = np.zeros((P, 2, 512), dtype=np.float32)
    for s_ in range(2):
        mask_t[:, s_] = 1.0 * (t_idx <= j_idx - 128 * s_)
    iota_t = np.broadcast_to(
        (np.arange(T, dtype=np.float32) + 1.0) * np.float32(LAM), (P, T)
    ).copy()

    in_maps = []
    for c in range(N_CORES):
        b, h = c // 2, c % 2
        qrows = slice(h * EQ, (h + 1) * EQ)
        krows = slice(Q_DIM + h * EKV, Q_DIM + (h + 1) * EKV)
        vrows = slice(Q_DIM + KV_DIM + h * EKV, Q_DIM + (h + 1) * EKV + KV_DIM)
        wq_r = _re3(np.ascontiguousarray(w_qkv[qrows].T * WSCALE))
        wq_r4 = np.ascontiguousarray(
            wq_r.reshape(P, KSUB, H_Q, P).transpose(2, 0, 1, 3)
        ).astype(NPF8)  # [H_Q, P, 16, 128]
        xt = np.ascontiguousarray(x[b].T)
        in_maps.append(
            {
                "x16t": xt.astype(NPBF),
                "x8t": xt.astype(NPF8),
                "wq8": wq_r4,
                "wk8": _re3(np.ascontiguousarray(w_qkv[krows].T * WSCALE)).astype(
                    NPF8
                ),
                "wv16": _re3(np.ascontiguousarray(w_qkv[vrows].T)).astype(NPBF),
                "wo16": _re3(
                    np.ascontiguousarray(w_o[:, h * EQ : (h + 1) * EQ].T)
                ).reshape(P, H_Q, D).astype(NPBF),
                "cos_t": cos_t,
                "sin_t": sin_t,
                "ones_m": ones_m,
                "pswap": pswap,
                "ident": ident,
                "mask_t": mask_t,
                "iota_t": iota_t,
            }
        )
    return in_maps


def _gather(results):
    out = np.empty((B, T, D), dtype=np.float32)
    for b in range(B):
        acc = np.asarray(results[2 * b]["out_t"], np.float32) + np.asarray(
            results[2 * b + 1]["out_t"], np.float32
        )
        out[b] = acc.T
    return out


_NC_CACHE = []


def _get_module():
    if not _NC_CACHE:
        _NC_CACHE.append(_build_module())
    return _NC_CACHE[0]


def kernel(x, w_qkv, w_o):
    nc = _get_module()
    in_maps = _host_inputs(x, w_qkv, w_o)
    results = bass2jax.run_bass_via_pjrt(nc, in_maps, n_cores=N_CORES)
    return _gather(results)


# revision 54
# speedup vs baseline: 1.1291x; 1.0123x over previous
"""Causal GQA attention block (QK L2-norm + RoPE) for 8 trn2 NeuronCores.

Sharding: tensor-parallel over head-halves (2) x data-parallel over batch (4).
Core c handles batch c//2 and heads [h*8, h*8+8) with h = c%2.

Fast-path design:
  - QK projection in fp8e4m3 DoubleRow (256-deep contraction per pass).
    Weights pre-scaled by 64; the L2 norm absorbs the scale exactly.
  - Linear softmax: with QK-norm the logits are bounded by +-0.0884, so
    exp(p) ~ 1 + p.  att = 1 + SCALE*s splits into an exact base (cumsum
    tables) plus an fp8 correction d8 (x) v8 run as DoubleRow matmuls.
  - Base numerator lam*cumsum(V^T) comes from a DVE prefix scan over
    PE-transposed V (no triangle matmuls); the denominator row-sum uses
    sum_k s_jk = q_j . cumsum(K)_j (one DVE mul + one ones-matmul per
    q-tile) instead of per-pair fp8 sum matmuls.
  - Diagonal score/AV matmuls run only over their causal column ranges.
  - Phase-1 norm/rope chain is pipelined across iterations: the ssq/swap
    matmuls of iteration i-1 are emitted after the raw matmuls of i, so
    the tensor queue never waits on the scalar/vector chain.
  - Phase-2 processes q-tiles in interleaved pairs (3,0) then (2,1) so
    the vector-heavy small tiles hide under the tensor-heavy large ones.
All scale factors are folded into host tables (lambda = 1/SCALE).
"""

import numpy as np
import ml_dtypes

import concourse.mybir as mybir
import concourse.tile as tile
from concourse import bacc
from concourse import bass2jax

F32 = mybir.dt.float32
F32R = mybir.dt.float32r
BF16 = mybir.dt.bfloat16
F8 = mybir.dt.float8e4
AF = mybir.ActivationFunctionType
ALU = mybir.AluOpType
PM = mybir.MatmulPerfMode

NPF8 = ml_dtypes.float8_e4m3
NPBF = ml_dtypes.bfloat16

P = 128
B, T, D = 4, 2048, 2048
N_HEADS, HEAD_DIM, N_KV = 16, 128, 4
Q_DIM = N_HEADS * HEAD_DIM          # 2048
KV_DIM = N_KV * HEAD_DIM            # 512
H_Q = 8                             # q heads per core
H_KV = 2                            # kv heads per core
EQ = H_Q * HEAD_DIM                 # 1024 q features per core
EKV = H_KV * HEAD_DIM               # 256
SCALE = 0.08838834764831845
LAM = 1.0 / SCALE
WSCALE = 64.0                       # fp8 pre-scale on w_qk (norm absorbs it)
THETA = 10000.0

KSUB = D // P                       # 16 contraction subtiles
KPAIR = KSUB // 2                   # 8 DoubleRow pairs
N_CORES = 8
TT_HALF = T // 2                    # 1024, phase-1 token half
NT512 = T // 512                    # 4 512-token q tiles
NTB = T // P                        # 16 128-token blocks


def _build_module():
    nc = bacc.Bacc("TRN2", target_bir_lowering=False, debug=False)

    x16t = nc.dram_tensor("x16t", [D, T], BF16, kind="ExternalInput")
    x8t = nc.dram_tensor("x8t", [D, T], F8, kind="ExternalInput")
    wq8 = nc.dram_tensor("wq8", [H_Q, P, KSUB, P], F8, kind="ExternalInput")
    wk8 = nc.dram_tensor("wk8", [P, KSUB, EKV], F8, kind="ExternalInput")
    wv16 = nc.dram_tensor("wv16", [P, KSUB, EKV], BF16, kind="ExternalInput")
    wo16 = nc.dram_tensor("wo16", [P, H_Q, D], BF16, kind="ExternalInput")
    cos_t = nc.dram_tensor("cos_t", [P, T], BF16, kind="ExternalInput")
    sin_t = nc.dram_tensor("sin_t", [P, T], BF16, kind="ExternalInput")
    ones_m = nc.dram_tensor("ones_m", [P, P], BF16, kind="ExternalInput")
    pswap = nc.dram_tensor("pswap", [P, P], BF16, kind="ExternalInput")
    ident = nc.dram_tensor("ident", [P, P], BF16, kind="ExternalInput")
    mask_t = nc.dram_tensor("mask_t", [P, 2, 512], F32R, kind="ExternalInput")
    iota_t = nc.dram_tensor("iota_t", [P, T], F32, kind="ExternalInput")
    out_t = nc.dram_tensor("out_t", [D, T], BF16, kind="ExternalOutput")

    with tile.TileContext(nc) as tc:
        with (
            tc.tile_pool(name="persist", bufs=1) as persist,
            tc.tile_pool(name="kv_persist", bufs=1) as kvp,
            tc.tile_pool(name="att_sb", bufs=8) as att_sb,
        ):
            ones_sb = persist.tile([P, P], BF16)
            psw_sb = persist.tile([P, P], BF16)
            id_sb = persist.tile([P, P], BF16)
            mask_sb = persist.tile([P, 2, 512], F32R)
            iota_sb = persist.tile([P, T], F32)
            nc.gpsimd.dma_start(ones_sb[:], ones_m.ap())
            nc.gpsimd.dma_start(psw_sb[:], pswap.ap())
            nc.gpsimd.dma_start(id_sb[:], ident.ap())
            nc.gpsimd.dma_start(mask_sb[:], mask_t.ap())
            nc.gpsimd.dma_start(iota_sb[:], iota_t.ap())
            k_sb = kvp.tile([P, H_KV, T], BF16)     # roped+normed K^T slabs
            v8_sb = kvp.tile([P, NTB, EKV], F8)     # V in [t, e] layout, fp8
            vT_sb = kvp.tile([P, H_KV, T], BF16)    # V^T in [e, t]
            q_all = kvp.tile([P, H_Q, T], BF16)     # Q resident in SBUF
            kc_sb = kvp.tile([P, H_KV, T], BF16)    # cumsum(K) along tokens
            vc_sb = kvp.tile([P, H_KV, T], F32)     # cumsum(V^T)

            # ---------------- phase 1: qkv proj + L2 norm + rope ----------
            with (
                tc.tile_pool(name="xres", bufs=1) as xres,
                tc.tile_pool(name="wstream", bufs=3) as wstream,
                tc.tile_pool(name="wvres", bufs=1) as wvres,
                tc.tile_pool(name="p1tmp", bufs=3) as p1tmp,
                tc.tile_pool(name="vstage", bufs=2) as vstage,
                tc.tile_pool(name="trig", bufs=1) as trig,
                tc.tile_pool(name="pp", bufs=4, space="PSUM") as pp,
                tc.tile_pool(name="pssq", bufs=2, space="PSUM") as pssq,
                tc.tile_pool(name="psw", bufs=2, space="PSUM") as psw,
            ):
                cos_sb = trig.tile([P, T], BF16)
                sin_sb = trig.tile([P, T], BF16)
                wv_sb = wvres.tile([P, KSUB, EKV], BF16)
                wk_sb = wvres.tile([P, KSUB, EKV], F8, name="wk_sb")
                nc.sync.dma_start(wk_sb[:, :, :P], wk8.ap()[:, :, :P])
                nc.scalar.dma_start(wk_sb[:, :, P:], wk8.ap()[:, :, P:])

                # two-stage deferred tail of the norm/rope chain: the norm
                # DVE chain runs one iteration behind the raw matmuls, the
                # swap matmul + rope combine three behind, so the tensor
                # queue never waits on a same-iteration cross-engine result
                p1q = []
                p2q = []

                def emit_part1(c):
                    raw_ps, sq, dest, tg = c
                    ssq_ps = pssq.tile([P, 512], F32, tag="ssq")
                    nc.tensor.matmul(
                        ssq_ps[:], ones_sb[:], sq[:], start=True, stop=True
                    )
                    s_sb = p1tmp.tile([P, 512], F32, tag="t2")
                    nc.scalar.activation(s_sb[:], ssq_ps[:], AF.Sqrt)
                    r_sb = p1tmp.tile([P, 512], F32, tag="t3")
                    nc.vector.reciprocal_approx_fast(r_sb[:], s_sb[:])
                    qn = p1tmp.tile([P, 512], BF16, tag="t4")
                    nc.vector.tensor_mul(qn[:], raw_ps[:], r_sb[:])
                    ys = p1tmp.tile([P, 512], BF16, tag="t5")
                    nc.vector.tensor_mul(ys[:], qn[:], sin_sb[:, tg : tg + 512])
                    qc = p1tmp.tile([P, 512], BF16, tag="t6")
                    nc.gpsimd.tensor_mul(qc[:], qn[:], cos_sb[:, tg : tg + 512])
                    return (ys, qc, dest)

                def emit_part2(c):
                    ys, qc, dest = c
                    sw_ps = psw.tile([P, 512], F32, tag="sw")
                    nc.tensor.matmul(
                        sw_ps[:], psw_sb[:], ys[:], start=True, stop=True
                    )
                    nc.vector.tensor_add(dest, sw_ps[:], qc[:])

                def step():
                    if len(p1q) > 1:
                        p2q.append(emit_part1(p1q.pop(0)))
                    if len(p2q) > 2:
                        emit_part2(p2q.pop(0))

                def flush():
                    while p1q:
                        p2q.append(emit_part1(p1q.pop(0)))
                    while p2q:
                        emit_part2(p2q.pop(0))

                for th in range(2):
                    t0 = th * TT_HALF
                    x8_sb = [
                        xres.tile(
                            [P, 2, TT_HALF], F8, tag=f"x8_{kp}", name=f"x8_{kp}"
                        )
                        for kp in range(KPAIR)
                    ]
                    x16_sb = [
                        xres.tile([P, TT_HALF], BF16, tag=f"x16_{ks}", name=f"x16_{ks}")
                        for ks in range(KSUB)
                    ]
                    xr16 = x16t.ap()[:, t0 : t0 + TT_HALF].rearrange(
                        "(ks p) t -> p ks t", p=P
                    )
                    xr8 = x8t.ap()[:, t0 : t0 + TT_HALF].rearrange(
                        "(ks p) t -> p ks t", p=P
                    )
                    # x8 first (pair 0 gates the first raw matmul); x16 on the
                    # gpsimd queue (only needed by the V projection).  One 2D
                    # DMA per ks slab (a fused [P,2,T/2] copy would be 3D).
                    for kp in range(KPAIR):
                        eng = nc.sync if kp % 2 == 0 else nc.scalar
                        for j in range(2):
                            eng.dma_start(x8_sb[kp][:, j], xr8[:, 2 * kp + j])
                    if th == 0:
                        nc.gpsimd.dma_start(cos_sb[:], cos_t.ap())
                        nc.gpsimd.dma_start(sin_sb[:], sin_t.ap())
                    for ks in range(KSUB):
                        nc.gpsimd.dma_start(x16_sb[ks][:], xr16[:, ks])
                    if th == 0:
                        nc.gpsimd.dma_start(wv_sb[:], wv16.ap())

                    def proj_norm_rope(es):
                        """project feature block es (fp8 DoubleRow), norm, rope"""
                        if es < H_Q:
                            w_sb = wstream.tile([P, KSUB, P], F8, tag="w")
                            nc.sync.dma_start(w_sb[:], wq8.ap()[es])
                            w_use = w_sb
                        else:
                            w_use = wk_sb
                        for tt in range(2):
                            tg = t0 + tt * 512
                            sl = slice(tt * 512, (tt + 1) * 512)
                            raw_ps = pp.tile([P, 512], F32, tag="raw")
                            for kp in range(KPAIR):
                                if es < H_Q:
                                    lhs = w_use[:, 2 * kp : 2 * kp + 2, :]
                                else:
                                    e0 = (es - H_Q) * P
                                    lhs = w_use[:, 2 * kp : 2 * kp + 2, e0 : e0 + P]
                                nc.tensor.matmul(
                                    raw_ps[:],
                                    lhs,
                                    x8_sb[kp][:, :, sl],
                                    start=(kp == 0),
                                    stop=(kp == KPAIR - 1),
                                    perf_mode=PM.DoubleRow,
                                )
                            sq = p1tmp.tile([P, 512], BF16, tag="t1")
                            nc.scalar.activation(sq[:], raw_ps[:], AF.Square)
                            if es < H_Q:
                                dest = q_all[:, es, tg : tg + 512]
                            else:
                                dest = k_sb[:, es - H_Q, tg : tg + 512]
                            p1q.append((raw_ps, sq, dest, tg))
                            step()

                    # K first so downstream work can start earliest, then Q
                    for es in (H_Q, H_Q + 1):
                        proj_norm_rope(es)
                    if th == 1:
                        # K complete: start its prefix scan now so phase 2
                        # isn't gated on the end of the vector queue
                        flush()
                        for kvi in range(H_KV):
                            nc.vector.tensor_tensor_scan(
                                kc_sb[:, kvi],
                                k_sb[:, kvi],
                                k_sb[:, kvi],
                                0.0,
                                ALU.add,
                                ALU.bypass,
                            )
                    for es in range(H_Q):
                        proj_norm_rope(es)
                    flush()
                    # V projection + fp8 cast + PE transpose for the scan
                    for tb in range(TT_HALF // P):
                        tbg = th * (TT_HALF // P) + tb
                        v_ps = pp.tile([P, EKV], F32, tag="raw")
                        for ks in range(KSUB):
                            nc.tensor.matmul(
                                v_ps[:],
                                x16_sb[ks][:, tb * P : (tb + 1) * P],
                                wv_sb[:, ks],
                                start=(ks == 0),
                                stop=(ks == KSUB - 1),
                            )
                        vst = vstage.tile([P, EKV], BF16, tag="vs")
                        nc.scalar.copy(vst[:], v_ps[:])
                        nc.scalar.copy(v8_sb[:, tbg], v_ps[:])
                        for kvi in range(H_KV):
                            tp_ps = pssq.tile([P, P], BF16, tag="ssq")
                            nc.tensor.transpose(
                                tp_ps[:],
                                vst[:, kvi * P : (kvi + 1) * P],
                                id_sb[:],
                            )
                            nc.scalar.copy(
                                vT_sb[:, kvi, tbg * P : (tbg + 1) * P], tp_ps[:]
                            )
                # V^T complete: full-length prefix scan
                for kvi in range(H_KV):
                    nc.vector.tensor_tensor_scan(
                        vc_sb[:, kvi],
                        vT_sb[:, kvi],
                        vT_sb[:, kvi],
                        0.0,
                        ALU.add,
                        ALU.bypass,
                    )

            # ------- phase 2: attention + output projection per q-tile ----
            with (
                tc.tile_pool(name="wores", bufs=1) as wores,
                tc.tile_pool(name="p2tmp", bufs=4) as p2tmp,
                tc.tile_pool(name="oall", bufs=2) as oall,
                tc.tile_pool(name="fout", bufs=3) as fout,
                tc.tile_pool(name="psc", bufs=4, space="PSUM") as psc,
                tc.tile_pool(name="pav", bufs=2, space="PSUM") as pav,
                tc.tile_pool(name="psum2", bufs=2, space="PSUM") as psum2,
            ):
                wo_sb = wores.tile([P, H_Q, D], BF16)
                for ei in range(H_Q):
                    nc.gpsimd.dma_start(wo_sb[:, ei], wo16.ap()[:, ei])

                def attn_head(qt, hd, o_all):
                    q0 = qt * 512
                    nkb = (qt + 1) * 4
                    npair = nkb // 2
                    kvi = hd // 4
                    o_ps = pav.tile([P, 512], F32, tag="av")

                    # denominator numerand: q . Kc (exact causal row-sum)
                    qkc = p2tmp.tile([P, 512], BF16, tag="qk")
                    nc.gpsimd.tensor_mul(
                        qkc[:],
                        q_all[:, hd, q0 : q0 + 512],
                        kc_sb[:, kvi, q0 : q0 + 512],
                    )
                    # all scores + casts first (per-block psum tiles, 4-deep
                    # rotation hides cast latency), then all AV matmuls
                    d8s = []
                    for pj in range(npair - 2):
                        d8 = att_sb.tile([P, 2, 512], F8, tag="att")
                        for j in range(2):
                            kb = 2 * pj + j
                            sc_ps = psc.tile([P, 512], F32, tag="sc")
                            nc.tensor.matmul(
                                sc_ps[:],
                                k_sb[:, kvi, kb * P : (kb + 1) * P],
                                q_all[:, hd, q0 : q0 + 512],
                                start=True,
                                stop=True,
                            )
                            nc.scalar.activation(d8[:, j], sc_ps[:], AF.Copy)
                        d8s.append(d8)
                    # diagonal pair A (rel blocks 0,1): masked; block 1 only
                    # needs cols >= 128 (the masked-off psum region holds
                    # stale-but-finite scores from earlier full writes)
                    d8a = att_sb.tile([P, 2, 512], F8, tag="att")
                    for j in range(2):
                        sc_ps = psc.tile([P, 512], F32, tag="sc")
                        nc.tensor.matmul(
                            sc_ps[:, 128 * j :],
                            k_sb[:, kvi, (nkb - 4 + j) * P : (nkb - 3 + j) * P],
                            q_all[:, hd, q0 + 128 * j : q0 + 512],
                            start=True,
                            stop=True,
                        )
                        nc.vector.tensor_mul(d8a[:, j], sc_ps[:], mask_sb[:, j])
                    # diagonal pair B (rel blocks 2,3): cols [256:512)
                    d8b = att_sb.tile([P, 2, 256], F8, tag="attb")
                    for j in range(2):
                        sc_ps = psc.tile([P, 256], F32, tag="sc")
                        nc.tensor.matmul(
                            sc_ps[:, 128 * j :],
                            k_sb[:, kvi, (nkb - 2 + j) * P : (nkb - 1 + j) * P],
                            q_all[:, hd, q0 + 256 + 128 * j : q0 + 512],
                            start=True,
                            stop=True,
                        )
                        nc.vector.tensor_mul(
                            d8b[:, j], sc_ps[:], mask_sb[:, j, :256]
                        )
                    for pj in range(npair - 2):
                        kb0 = 2 * pj
                        nc.tensor.matmul(
                            o_ps[:],
                            v8_sb[:, kb0 : kb0 + 2, kvi * P : (kvi + 1) * P],
                            d8s[pj][:],
                            start=(pj == 0),
                            stop=False,
                            perf_mode=PM.DoubleRow,
                        )
                    nc.tensor.matmul(
                        o_ps[:],
                        v8_sb[:, nkb - 4 : nkb - 2, kvi * P : (kvi + 1) * P],
                        d8a[:],
                        start=(npair == 2),
                        stop=True,
                        perf_mode=PM.DoubleRow,
                    )
                    nc.tensor.matmul(
                        o_ps[:, 256:],
                        v8_sb[:, nkb - 2 : nkb, kvi * P : (kvi + 1) * P],
                        d8b[:],
                        start=False,
                        stop=True,
                        perf_mode=PM.DoubleRow,
                        skip_group_check=True,
                    )
                    # denominator: iota + sum(q . Kc)
                    den_ps = psum2.tile([P, 512], F32, tag="sum")
                    nc.tensor.matmul(
                        den_ps[:], ones_sb[:], qkc[:], start=True, stop=True
                    )
                    den = p2tmp.tile([P, 512], F32, tag="dn")
                    nc.vector.tensor_add(
                        den[:], den_ps[:], iota_sb[:, q0 : q0 + 512]
                    )
                    rs = p2tmp.tile([P, 512], F32, tag="rs")
                    nc.vector.reciprocal_approx_fast(rs[:], den[:])
                    # finalize: (corr + lam*Vc) / den
                    fa = p2tmp.tile([P, 512], BF16, tag="fa")
                    nc.vector.scalar_tensor_tensor(
                        fa[:],
                        vc_sb[:, kvi, q0 : q0 + 512],
                        float(LAM),
                        o_ps[:],
                        op0=ALU.mult,
                        op1=ALU.add,
                    )
                    nc.gpsimd.tensor_mul(o_all[:, hd], fa[:], rs[:])

                def o_proj(qt, o_all):
                    q0 = qt * 512
                    for eo in range(D // P):
                        f_ps = psum2.tile([P, 512], F32, tag="sum")
                        for ei in range(H_Q):
                            nc.tensor.matmul(
                                f_ps[:],
                                wo_sb[:, ei, eo * P : (eo + 1) * P],
                                o_all[:, ei],
                                start=(ei == 0),
                                stop=(ei == H_Q - 1),
                            )
                        f_sb = fout.tile([P, 512], BF16, tag="fo")
                        nc.scalar.copy(f_sb[:], f_ps[:])
                        nc.sync.dma_start(
                            out_t.ap()[eo * P : (eo + 1) * P, q0 : q0 + 512],
                            f_sb[:],
                        )

                # interleave a tensor-heavy and a vector-heavy q-tile so the
                # small tiles' DVE chains hide under the big tiles' matmuls
                for qta, qtb in ((3, 0), (2, 1)):
                    oa = oall.tile([P, H_Q, 512], BF16, tag="oa", name="oa_a")
                    ob = oall.tile([P, H_Q, 512], BF16, tag="oa", name="oa_b")
                    for hd in range(H_Q):
                        attn_head(qta, hd, oa)
                        attn_head(qtb, hd, ob)
                    o_proj(qta, oa)
                    o_proj(qtb, ob)

    nc.compile()
    return nc


def _re3(a):
    """[K, E] -> [P, K//P, E] host rearrange for contiguous weight DMAs."""
    return np.ascontiguousarray(a.reshape(-1, P, a.shape[1]).transpose(1, 0, 2))


def _host_inputs(x, w_qkv, w_o):
    """Build the 8 per-core input maps from full inputs."""
    x = np.asarray(x, dtype=np.float32)
    w_qkv = np.asarray(w_qkv, dtype=np.float32)
    w_o = np.asarray(w_o, dtype=np.float32)

    half = HEAD_DIM // 2
    inv_freq = 1.0 / (
        THETA ** (np.arange(0, HEAD_DIM, 2, dtype=np.float32) / HEAD_DIM)
    )
    ang = np.arange(T, dtype=np.float32)[:, None] * inv_freq[None, :]  # [T, 64]
    cos = np.cos(ang).T.astype(np.float32)  # [64, T]
    sin = np.sin(ang).T.astype(np.float32)
    cos_t = np.ascontiguousarray(np.concatenate([cos, cos], axis=0)).astype(NPBF)
    sin_t = np.ascontiguousarray(np.concatenate([sin, sin], axis=0)).astype(NPBF)

    ones_m = np.ones((P, P), dtype=np.float32).astype(NPBF)
    pswap = np.zeros((P, P), dtype=np.float32)
    for p in range(half):
        pswap[p, p + half] = 1.0    # out[m=p+64] += ys[p]
        pswap[p + half, p] = -1.0   # out[m=p]    -= ys[p+64]
    pswap = pswap.astype(NPBF)
    ident = np.eye(P, dtype=np.float32).astype(NPBF)

    t_idx = np.arange(P, dtype=np.float32)[:, None]        # key within block
    j_idx = np.arange(512, dtype=np.float32)[None, :]      # query col
    mask_t = np.zeros((P, 2, 512), dtype=np.float32)
    for s_ in range(2):
        mask_t[:, s_] = 1.0 * (t_idx <= j_idx - 128 * s_)
    iota_t = np.broadcast_to(
        (np.arange(T, dtype=np.float32) + 1.0) * np.float32(LAM), (P, T)
    ).copy()

    in_maps = []
    for c in range(N_CORES):
        b, h = c // 2, c % 2
        qrows = slice(h * EQ, (h + 1) * EQ)
        krows = slice(Q_DIM + h * EKV, Q_DIM + (h + 1) * EKV)
        vrows = slice(Q_DIM + KV_DIM + h * EKV, Q_DIM + (h + 1) * EKV + KV_DIM)
        wq_r = _re3(np.ascontiguousarray(w_qkv[qrows].T * WSCALE))
        wq_r4 = np.ascontiguousarray(
            wq_r.reshape(P, KSUB, H_Q, P).transpose(2, 0, 1, 3)
        ).astype(NPF8)  # [H_Q, P, 16, 128]
        xt = np.ascontiguousarray(x[b].T)
        in_maps.append(
            {
                "x16t": xt.astype(NPBF),
                "x8t": xt.astype(NPF8),
                "wq8": wq_r4,
                "wk8": _re3(np.ascontiguousarray(w_qkv[krows].T * WSCALE)).astype(
                    NPF8
                ),
                "wv16": _re3(np.ascontiguousarray(w_qkv[vrows].T)).astype(NPBF),
                "wo16": _re3(
                    np.ascontiguousarray(w_o[:, h * EQ : (h + 1) * EQ].T)
                ).reshape(P, H_Q, D).astype(NPBF),
                "cos_t": cos_t,
                "sin_t": sin_t,
                "ones_m": ones_m,
                "pswap": pswap,
                "ident": ident,
                "mask_t": mask_t,
                "iota_t": iota_t,
            }
        )
    return in_maps


def _gather(results):
    out = np.empty((B, T, D), dtype=np.float32)
    for b in range(B):
        acc = np.asarray(results[2 * b]["out_t"], np.float32) + np.asarray(
            results[2 * b + 1]["out_t"], np.float32
        )
        out[b] = acc.T
    return out


_NC_CACHE = []


def _get_module():
    if not _NC_CACHE:
        _NC_CACHE.append(_build_module())
    return _NC_CACHE[0]


def kernel(x, w_qkv, w_o):
    nc = _get_module()
    in_maps = _host_inputs(x, w_qkv, w_o)
    results = bass2jax.run_bass_via_pjrt(nc, in_maps, n_cores=N_CORES)
    return _gather(results)
